# revision 7
# baseline (speedup 1.0000x reference)
"""Trainium2 Bass kernel for nn_Block_71554155151896 (GNN message passing).

Sharding: edges sorted by dst on host, split into 8 contiguous 128-aligned
dst-node ranges (one per core). All edges of a node live on one core, so
segment softmax + scatter-sum are core-local (no collectives). The device
does all model math (MLPs, LN, softmax, tensor product, gathers, one-hot
matmul scatter); the host only does index prep / padding / permutation.

Layout: feature-on-partition (fep) [d, edges] for the matmul chain, bf16
matmul inputs with f32 PSUM accumulation. Edges processed per 128-node
window (NWSUB=9 sub-chunks of 128 edges, G=3 matmul groups of 384).
"""
import sys
sys.path.insert(0, "/opt/trn_rl_repo")
import os
import numpy as np
import ml_dtypes
from contextlib import ExitStack

import concourse.bass as bass
import concourse.tile as tile
from concourse import bacc, mybir
from concourse.bass_utils import run_bass_kernel_spmd

bf16 = ml_dtypes.bfloat16
F32 = mybir.dt.float32
BF = mybir.dt.bfloat16
I32 = mybir.dt.int32

N, E, D, S, H, L = 50000, 400000, 64, 9, 4, 64
DOUT = 64
NCORES = 8
P = 128
NWSUB = 9
WE = NWSUB * P        # 1152 edges / window
G = 3
CG = WE // G          # 384
SPG = NWSUB // G      # 3

AF = mybir.ActivationFunctionType
OP = mybir.AluOpType


def _to_bf(x):
    return np.asarray(x, np.float32).astype(bf16)


# ------------------------------------------------------------------ host prep
def preprocess(inputs):
    src = np.asarray(inputs['edge_src'])
    dst = np.asarray(inputs['edge_dst'])
    elen = np.asarray(inputs['edge_length_embedding'])
    sh = np.asarray(inputs['edge_sh'])
    node = np.asarray(inputs['node_in'])

    order = np.argsort(dst, kind='stable')
    dst_s = dst[order]

    bounds = [0]
    for k in range(1, NCORES):
        target = k * E // NCORES
        bounds.append(min((int(dst_s[target]) + 127) // 128 * 128, N))
    bounds.append(((N + 127) // 128) * 128)
    W_max = max((bounds[k + 1] - bounds[k] + 127) // 128 for k in range(NCORES))

    cores = []
    for k in range(NCORES):
        n0, n1 = bounds[k], bounds[k + 1]
        lo = np.searchsorted(dst_s, n0, 'left')
        hi = np.searchsorted(dst_s, min(n1, N), 'left') if n1 <= N else E
        eidx = order[lo:hi]
        dstk = dst[eidx] - n0
        W = (n1 - n0 + 127) // 128

        eid = np.full((W_max, WE), -1, np.int64)
        for w in range(W):
            ids = eidx[(dstk // 128) == w]
            assert len(ids) <= WE, f"window overflow {len(ids)}"
            eid[w, :len(ids)] = ids
        eid = eid.reshape(-1)
        valid = eid >= 0
        eid_c = np.where(valid, eid, 0)
        Ep = eid.size

        uniq, src_rel = np.unique(src[eid_c], return_inverse=True)
        src_rel = np.where(valid, src_rel, len(uniq)).astype(np.int32)
        dst_rel = np.where(valid, dst[eid_c] - n0, 0)
        rel_in_win = dst_rel % 128

        ar = np.arange(128)
        oh_eop = ((rel_in_win[:, None] == ar[None, :]) &
                  valid[:, None]).astype(np.float32)           # [Ep, 128]
        # nop per window: [W_max*128 nodes, WE]
        oh_nop = np.zeros((W_max * 128, WE), bf16)
        for w in range(W_max):
            blk = oh_eop[w * WE:(w + 1) * WE]                  # [WE, 128]
            oh_nop[w * 128:(w + 1) * 128, :] = _to_bf(blk.T)

        table_src = np.zeros((len(uniq) + 128, 64), bf16)
        table_src[:len(uniq)] = _to_bf(node[uniq])

        nreal = min(n1, N) - n0
        dstfeat = np.zeros((W_max * 128, D), bf16)
        dstfeat[:nreal] = _to_bf(node[n0:n0 + nreal])

        elen_T = np.zeros((D, Ep), bf16)
        elen_T[:, valid] = _to_bf(elen[eid_c]).T[:, valid]
        sh_T = np.zeros((S, Ep), bf16)
        sh_T[:, valid] = _to_bf(sh[eid_c]).T[:, valid]

        srcidx = np.zeros((W_max, 128, NWSUB), np.int32)
        sr = src_rel.reshape(W_max, NWSUB, 128)
        srcidx[:] = np.transpose(sr, (0, 2, 1))

        cores.append(dict(
            n0=n0, n1=n1, W=W, eid=eid, valid=valid,
            elenT=elen_T, shT=sh_T, oheop=_to_bf(oh_eop),
            ohnop=oh_nop, srcidx=srcidx,
            table_src=table_src, dstfeat=dstfeat,
        ))
    NSRC = max(c['table_src'].shape[0] for c in cores)
    for c in cores:
        t = c['table_src']
        if t.shape[0] < NSRC:
            c['table_src'] = np.vstack(
                [t, np.zeros((NSRC - t.shape[0], 64), bf16)])
    return cores, dict(W_max=W_max, NSRC=NSRC, bounds=bounds)


def pack_weights(params):
    cols = {}
    buf = []
    pos = 0

    def add(name, mat):
        nonlocal pos
        mat = np.asarray(mat, np.float32)
        m = np.zeros((128, mat.shape[1]), np.float32)
        m[:mat.shape[0]] = mat
        buf.append(m)
        cols[name] = (pos, mat.shape[1])
        pos += mat.shape[1]

    def lhsT_aug(Wm, bm=None):
        Wm = np.asarray(Wm, np.float32)
        rows = [Wm] + ([np.asarray(bm, np.float32)[None, :]] if bm is not None else [])
        Wb = np.vstack(rows)
        return np.concatenate([Wb, Wb.mean(1, keepdims=True)], axis=1)

    p = params
    a, sc, tp2 = p['alpha'], p['scalar'], p['tp2']
    Wp = np.asarray(p['W_pre'])
    add('Wsrc2', np.concatenate([Wp[0:64]] * 2, axis=1))       # [64, 128]
    add('Wdst2', np.concatenate([Wp[64:128]] * 2, axis=1))
    add('Welen2', np.concatenate([Wp[128:192]] * 2, axis=1))
    add('Wa1', lhsT_aug(a['W1'], a['b1']))                     # [65, 65]
    add('Wa2', lhsT_aug(a['W2'], a['b2']))
    add('Wa3', np.vstack([np.asarray(a['W3']),
                          np.asarray(a['b3'])[None, :]]))      # [65, 4]
    add('Wr1', np.asarray(tp2['W_r1']))
    add('Wr2', np.asarray(tp2['W_r2']))                        # [64, 256]
    Wtp = np.asarray(tp2['W_tp'])
    Wflat = np.transpose(Wtp, (1, 0, 2)).reshape(S * D, 256)
    Wflat_p = np.zeros((5 * 128, 256), np.float32)
    Wflat_p[:S * D] = Wflat
    for t in range(5):
        add(f'Wtp{t}', Wflat_p[t * 128:(t + 1) * 128])
    We = np.asarray(p['W_edge'])
    add('WeA', We[0:128]); add('WeB', We[128:256])
    add('Ws1', lhsT_aug(sc['W1'], sc['b1']))
    add('Ws2', lhsT_aug(sc['W2'], sc['b2']))
    add('Ws3', np.vstack([np.asarray(sc['W3']),
                          np.asarray(sc['b3'])[None, :]]))     # [65, 32]
    Wo = np.asarray(p['W_out'])
    add('WoA', Wo[0:128]); add('WoB', Wo[128:256])

    add('ones64', np.full((64, 1), 1.0 / 64.0, np.float32))
    gpatA = np.zeros((1, 128), np.float32); gpatA[0, 0:64] = 1.0
    gpatB = np.zeros((1, 128), np.float32); gpatB[0, 64:128] = 1.0
    add('gpatA', gpatA); add('gpatB', gpatB)
    e4 = np.zeros((4, 256), np.float32)
    for h in range(4):
        e4[h, h * 64:(h + 1) * 64] = 1.0
    add('E4a', e4[:, 0:128]); add('E4b', e4[:, 128:256])
    add('I4', np.eye(4, dtype=np.float32))
    add('I128', np.eye(128, dtype=np.float32))
    for t in range(5):
        sel = np.zeros((S, 128), np.float32)
        for half in range(2):
            srow = 2 * t + half
            if srow < S:
                sel[srow, half * 64:(half + 1) * 64] = 1.0
        add(f'SEL{t}', sel)
    return _to_bf(np.concatenate(buf, axis=1)), cols


def pack_biases(params):
    cols = {}
    buf = []
    tp2 = params['tp2']
    for name, vec in [('br1', tp2['b_r1']),
                      ('br2a', np.asarray(tp2['b_r2'])[0:128]),
                      ('br2b', np.asarray(tp2['b_r2'])[128:256]),
                      ('zero', np.zeros(128))]:
        v = np.zeros((128, 1), np.float32)
        vv = np.asarray(vec, np.float32).ravel()
        v[:len(vv), 0] = vv
        cols[name] = len(buf)
        buf.append(v)
    return np.concatenate(buf, axis=1), cols


# ------------------------------------------------------------------ graph
def build_graph(W_max, NSRC, NWIN, wpack, bpack):
    Wpk, wc = wpack
    bpk, bc = bpack
    Ep = W_max * WE
    nc = bacc.Bacc('TRN2', target_bir_lowering=False, debug=False,
                   num_devices=NCORES)

    d_wei = nc.declare_dram_parameter("wei", list(Wpk.shape), BF, isOutput=False)
    d_bia = nc.declare_dram_parameter("bia", list(bpk.shape), F32, isOutput=False)
    d_elen = nc.declare_dram_parameter("elenT", [D, Ep], BF, isOutput=False)
    d_sh = nc.declare_dram_parameter("shT", [S, Ep], BF, isOutput=False)
    d_oh = nc.declare_dram_parameter("oheop", [Ep, 128], BF, isOutput=False)
    d_ohn = nc.declare_dram_parameter("ohnop", [W_max * 128, WE], BF, isOutput=False)
    d_sidx = nc.declare_dram_parameter("srcidx", [W_max, 128, NWSUB], I32, isOutput=False)
    d_tsrc = nc.declare_dram_parameter("tsrc", [NSRC, 64], BF, isOutput=False)
    d_dstf = nc.declare_dram_parameter("dstfeat", [W_max * 128, D], BF, isOutput=False)

    d_esc = nc.declare_dram_parameter("esc", [32, Ep], F32, isOutput=True)
    d_nout = nc.declare_dram_parameter("nodeout", [W_max * 128, DOUT], F32, isOutput=True)

    def raw_act(out, in_, func, bias, scale=1.0):
        eng = nc.scalar
        inputs = [eng.lower_ap(in_), eng.lower_ap(bias),
                  mybir.ImmediateValue(dtype=F32, value=float(scale)),
                  mybir.ImmediateValue(dtype=F32, value=0.0)]
        return eng.add_instruction(mybir.InstActivation(
            name=nc.get_next_instruction_name(),
            func=func, ins=inputs, outs=[eng.lower_ap(out)]))

    with ExitStack() as ctx:
        tc = ctx.enter_context(tile.TileContext(nc))
        cp = ctx.enter_context(tc.tile_pool(name="const", bufs=1))
        wp = ctx.enter_context(tc.tile_pool(name="win", bufs=2))
        gp = ctx.enter_context(tc.tile_pool(name="grp", bufs=2))
        ps_mlp = ctx.enter_context(tc.tile_pool(name="psm", bufs=2, space="PSUM"))
        ps_ln = ctx.enter_context(tc.tile_pool(name="psl", bufs=2, space="PSUM"))
        ps_ln1 = ctx.enter_context(tc.tile_pool(name="psl1", bufs=1, space="PSUM"))
        ps_tp = ctx.enter_context(tc.tile_pool(name="pst", bufs=2, space="PSUM"))
        ps_big = ctx.enter_context(tc.tile_pool(name="psb", bufs=1, space="PSUM"))

        wei = cp.tile([128, Wpk.shape[1]], BF, tag="wei")
        nc.sync.dma_start(wei[:], d_wei[:])
        bia = cp.tile([128, bpk.shape[1]], F32, tag="bia")
        nc.sync.dma_start(bia[:], d_bia[:])

        def Wt(nm, rows=128):
            o, n = wc[nm]
            return wei[0:rows, o:o + n]

        def Bi(nm, rows=128):
            return bia[0:rows, bc[nm]:bc[nm] + 1]

        nwin = cp.tile([128, W_max, 68], BF, tag="nwin")
        nc.sync.dma_start(
            nwin[:, :, 0:64],
            d_dstf[:].rearrange("(w p) d -> p w d", p=128))

        def ln_silu(x_ps, mlp_tag):
            """x_ps PSUM [65(+), CG]: rows 0:64 = x (incl bias), row 64 = mean.
            Returns SBUF bf16 [65, CG]: rows 0:64 silu(LN(x)), row 64 ones."""
            sq = gp.tile([64, CG], BF, tag="lnsq")
            nc.scalar.activation(sq[:], x_ps[0:64, :], AF.Square)
            msq = ps_ln1.tile([1, CG], F32, tag="lnmsq", space="PSUM")
            nc.tensor.matmul(msq[:], lhsT=Wt('ones64', 64), rhs=sq[:],
                             start=True, stop=True)
            musq = gp.tile([1, CG], F32, tag="lnmusq")
            nc.scalar.activation(musq[:], x_ps[64:65, :], AF.Square)
            var = gp.tile([1, CG], F32, tag="lnvar")
            nc.vector.scalar_tensor_tensor(out=var[:], in0=msq[:], scalar=1e-6,
                                           in1=musq[:], op0=OP.add,
                                           op1=OP.subtract)
            rstd = gp.tile([1, CG], BF, tag="lnrstd")
            raw_act(rstd[:], var[:], AF.Abs_reciprocal_sqrt,
                    bias=Bi('zero', 1))
            murstd = gp.tile([1, CG], BF, tag="lnmurstd")
            nc.vector.tensor_tensor(out=murstd[:], in0=x_ps[64:65, :],
                                    in1=rstd[:], op=OP.mult)
            bcab = ps_ln.tile([128, CG], F32, tag="lnbcab", space="PSUM")
            nc.tensor.matmul(bcab[:], lhsT=Wt('gpatA', 1), rhs=rstd[:],
                             start=True, stop=False)
            nc.tensor.matmul(bcab[:], lhsT=Wt('gpatB', 1), rhs=murstd[:],
                             start=False, stop=True)
            bcsa = gp.tile([64, CG], BF, tag="lnbcsa")
            nc.scalar.copy(bcsa[:], bcab[0:64, :])
            bcsb = gp.tile([64, CG], BF, tag="lnbcsb")
            nc.scalar.copy(bcsb[:], bcab[64:128, :])
            t1 = gp.tile([64, CG], BF, tag="lnt1")
            nc.vector.tensor_tensor(out=t1[:], in0=x_ps[0:64, :],
                                    in1=bcsa[:], op=OP.mult)
            t2 = gp.tile([64, CG], BF, tag="lnt2")
            nc.vector.tensor_tensor(out=t2[:], in0=t1[:], in1=bcsb[:],
                                    op=OP.subtract)
            out = gp.tile([65, CG], BF, tag="ln_" + mlp_tag)
            nc.vector.memset(out[64:65, :], 1.0)
            nc.scalar.activation(out[0:64, :], t2[:], AF.Silu)
            return out

        for w in range(NWIN):
            e0 = w * WE
            elw = wp.tile([65, WE], BF, tag="elw")
            nc.sync.dma_start(elw[0:64, :], d_elen[:, e0:e0 + WE])
            nc.vector.memset(elw[64:65, :], 1.0)
            shw = wp.tile([S, WE], BF, tag="shw")
            nc.sync.dma_start(shw[:], d_sh[:, e0:e0 + WE])
            ohw = wp.tile([128, NWSUB, 128], BF, tag="ohw")
            nc.sync.dma_start(
                ohw[:], d_oh[e0:e0 + WE, :].rearrange("(s p) n -> p s n", p=128))
            ohn = wp.tile([128, WE], BF, tag="ohn")
            nc.sync.dma_start(ohn[:], d_ohn[w * 128:(w + 1) * 128, :])

            # ---- pass A: alpha logits -> ex -> den ----
            exw = wp.tile([4, WE], BF, tag="exw")
            for g in range(G):
                c0 = g * CG
                x1 = ps_mlp.tile([65, CG], F32, tag="mlp", space="PSUM")
                nc.tensor.matmul(x1[:], lhsT=Wt('Wa1', 65),
                                 rhs=elw[:, c0:c0 + CG], start=True, stop=True)
                h1 = ln_silu(x1, "a1")
                x2 = ps_mlp.tile([65, CG], F32, tag="mlp", space="PSUM")
                nc.tensor.matmul(x2[:], lhsT=Wt('Wa2', 65), rhs=h1[:],
                                 start=True, stop=True)
                h2 = ln_silu(x2, "a2")
                lg = ps_mlp.tile([4, CG], F32, tag="mlp", space="PSUM")
                nc.tensor.matmul(lg[:], lhsT=Wt('Wa3', 65), rhs=h2[:],
                                 start=True, stop=True)
                nc.scalar.activation(exw[:, c0:c0 + CG], lg[:], AF.Exp)

            exe_ps = ps_big.tile([128, NWSUB * 4], F32, tag="big", space="PSUM")
            for s in range(NWSUB):
                nc.tensor.matmul(exe_ps[:, s * 4:(s + 1) * 4],
                                 lhsT=exw[:, s * P:(s + 1) * P],
                                 rhs=Wt('I4', 4), start=True, stop=True)
            exe = wp.tile([128, NWSUB * 4], BF, tag="exe")
            nc.scalar.copy(exe[:], exe_ps[:])

            den_ps = ps_ln1.tile([128, 4], F32, tag="lnmsq", space="PSUM")
            for s in range(NWSUB):
                nc.tensor.matmul(den_ps[:], lhsT=ohw[:, s, :],
                                 rhs=exe[:, s * 4:(s + 1) * 4],
                                 start=(s == 0), stop=(s == NWSUB - 1))
            dent = wp.tile([128, 4], F32, tag="dent")
            nc.vector.tensor_scalar(out=dent[:], in0=den_ps[:], scalar1=1e-12,
                                    scalar2=None, op0=OP.add)
            dent2 = wp.tile([128, 4], F32, tag="dent2")
            nc.vector.reciprocal(out=dent2[:], in_=dent[:])
            nc.scalar.copy(nwin[:, w, 64:68], dent2[:])

            # ---- src gather ----
            srcw = wp.tile([128, NWSUB, 64], BF, tag="srcw")
            sidx = wp.tile([128, NWSUB], I32, tag="sidx")
            nc.sync.dma_start(sidx[:], d_sidx[w])
            for s in range(NWSUB):
                nc.gpsimd.indirect_dma_start(
                    out=srcw[:, s, :], out_offset=None, in_=d_tsrc[:],
                    in_offset=bass.IndirectOffsetOnAxis(
                        ap=sidx[:, s:s + 1], axis=0))

            # ---- per-group value chain ----
            acc = wp.tile([128, DOUT], F32, tag="acc")
            nc.vector.memset(acc[:], 0.0)

            for g in range(G):
                c0 = g * CG
                # srcT fep
                srcT_ps = ps_ln.tile([64, CG], F32, tag="lnbcab", space="PSUM")
                for j in range(SPG):
                    s = g * SPG + j
                    nc.tensor.matmul(srcT_ps[:, j * P:(j + 1) * P],
                                     lhsT=srcw[:, s, :], rhs=Wt('I128'),
                                     start=True, stop=True)
                srcT = gp.tile([64, CG], BF, tag="srcT")
                nc.scalar.copy(srcT[:], srcT_ps[:])

                # dst features + recip den per edge (one MM per group)
                dstf_ps = ps_ln.tile([64, CG], F32, tag="lnbcab", space="PSUM")
                nc.tensor.matmul(dstf_ps[:], lhsT=nwin[:, w, 0:64],
                                 rhs=ohn[:, c0:c0 + CG], start=True, stop=True)
                dstT = gp.tile([64, CG], BF, tag="dstT")
                nc.scalar.copy(dstT[:], dstf_ps[:])
                dden_ps = ps_mlp.tile([4, CG], F32, tag="mlp", space="PSUM")
                nc.tensor.matmul(dden_ps[:], lhsT=nwin[:, w, 64:68],
                                 rhs=ohn[:, c0:c0 + CG], start=True, stop=True)
                dden = gp.tile([4, CG], BF, tag="dden")
                nc.scalar.copy(dden[:], dden_ps[:])

                # msg (stacked x2)
                msg_ps = ps_big.tile([128, CG], F32, tag="big", space="PSUM")
                nc.tensor.matmul(msg_ps[:], lhsT=Wt('Wsrc2', 64), rhs=srcT[:],
                                 start=True, stop=False)
                nc.tensor.matmul(msg_ps[:], lhsT=Wt('Wdst2', 64),
                                 rhs=dstT[:], start=False, stop=False)
                nc.tensor.matmul(msg_ps[:], lhsT=Wt('Welen2', 64),
                                 rhs=elw[0:64, c0:c0 + CG], start=False, stop=True)
                msg2 = gp.tile([128, CG], BF, tag="msg2")
                nc.scalar.copy(msg2[:], msg_ps[:])

                # alpha = ex * recip_den[dst]
                alpha4 = gp.tile([4, CG], BF, tag="alpha4")
                nc.vector.tensor_tensor(out=alpha4[:], in0=exw[:, c0:c0 + CG],
                                        in1=dden[:], op=OP.mult)
                aexp_ps = ps_tp.tile([128, CG], F32, tag="tp", space="PSUM")
                nc.tensor.matmul(aexp_ps[:], lhsT=Wt('E4a', 4), rhs=alpha4[:],
                                 start=True, stop=True)
                aexp_ps2 = ps_tp.tile([128, CG], F32, tag="tp", space="PSUM")
                nc.tensor.matmul(aexp_ps2[:], lhsT=Wt('E4b', 4), rhs=alpha4[:],
                                 start=True, stop=True)
                aexp = gp.tile([128, 2, CG], BF, tag="aexp")
                nc.scalar.copy(aexp[:, 0, :], aexp_ps[:])
                nc.scalar.copy(aexp[:, 1, :], aexp_ps2[:])

                # radial, radalpha = (radial + br2) * alphaexp
                r1_ps = ps_mlp.tile([64, CG], F32, tag="mlp", space="PSUM")
                nc.tensor.matmul(r1_ps[:], lhsT=Wt('Wr1', 64),
                                 rhs=elw[0:64, c0:c0 + CG], start=True, stop=True)
                r1 = gp.tile([64, CG], BF, tag="r1")
                nc.scalar.activation(r1[:], r1_ps[:], AF.Silu, bias=Bi('br1', 64))
                radal = gp.tile([128, 2, CG], BF, tag="radal")
                for half in range(2):
                    rad_ps = ps_big.tile([128, CG], F32, tag="big", space="PSUM")
                    nc.tensor.matmul(
                        rad_ps[:], lhsT=Wt('Wr2', 64)[:, half * 128:(half + 1) * 128],
                        rhs=r1[:], start=True, stop=True)
                    nc.vector.scalar_tensor_tensor(
                        out=radal[:, half, :], in0=rad_ps[:],
                        scalar=Bi('br2a' if half == 0 else 'br2b'),
                        in1=aexp[:, half, :], op0=OP.add, op1=OP.mult)

                # kron + tensor product
                tpA = ps_tp.tile([128, CG], F32, tag="tp", space="PSUM")
                tpB = ps_tp.tile([128, CG], F32, tag="tp", space="PSUM")
                for t in range(5):
                    shp_ps = ps_big.tile([128, CG], F32, tag="big", space="PSUM")
                    nc.tensor.matmul(shp_ps[:], lhsT=Wt(f'SEL{t}', S),
                                     rhs=shw[:, c0:c0 + CG], start=True, stop=True)
                    shp = gp.tile([128, CG], BF, tag="shp")
                    nc.scalar.copy(shp[:], shp_ps[:])
                    kro = gp.tile([128, CG], BF, tag="kro")
                    nc.vector.tensor_tensor(out=kro[:], in0=msg2[:], in1=shp[:],
                                            op=OP.mult)
                    nc.tensor.matmul(tpA[:], lhsT=Wt(f'Wtp{t}')[:, 0:128],
                                     rhs=kro[:], start=(t == 0), stop=(t == 4))
                    nc.tensor.matmul(tpB[:], lhsT=Wt(f'Wtp{t}')[:, 128:256],
                                     rhs=kro[:], start=(t == 0), stop=(t == 4))

                eo = gp.tile([128, 2, CG], BF, tag="eo")
                nc.vector.tensor_tensor(out=eo[:, 0, :], in0=tpA[:],
                                        in1=radal[:, 0, :], op=OP.mult)
                nc.vector.tensor_tensor(out=eo[:, 1, :], in0=tpB[:],
                                        in1=radal[:, 1, :], op=OP.mult)

                # edge scalar head
                xes_ps = ps_mlp.tile([64, CG], F32, tag="mlp", space="PSUM")
                nc.tensor.matmul(xes_ps[:], lhsT=Wt('WeA'), rhs=eo[:, 0, :],
                                 start=True, stop=False)
                nc.tensor.matmul(xes_ps[:], lhsT=Wt('WeB'), rhs=eo[:, 1, :],
                                 start=False, stop=True)
                xes = gp.tile([65, CG], BF, tag="xes")
                nc.vector.memset(xes[64:65, :], 1.0)
                nc.scalar.copy(xes[0:64, :], xes_ps[:])
                xs1 = ps_mlp.tile([65, CG], F32, tag="mlp", space="PSUM")
                nc.tensor.matmul(xs1[:], lhsT=Wt('Ws1', 65), rhs=xes[:],
                                 start=True, stop=True)
                hs1 = ln_silu(xs1, "s1")
                xs2 = ps_mlp.tile([65, CG], F32, tag="mlp", space="PSUM")
                nc.tensor.matmul(xs2[:], lhsT=Wt('Ws2', 65), rhs=hs1[:],
                                 start=True, stop=True)
                hs2 = ln_silu(xs2, "s2")
                esc_ps = ps_mlp.tile([32, CG], F32, tag="mlp", space="PSUM")
                nc.tensor.matmul(esc_ps[:], lhsT=Wt('Ws3', 65), rhs=hs2[:],
                                 start=True, stop=True)
                escs = gp.tile([32, CG], F32, tag="escs")
                nc.scalar.copy(escs[:], esc_ps[:])
                nc.sync.dma_start(d_esc[:, e0 + c0:e0 + c0 + CG], escs[:])

                # proj + scatter
                prT_ps = ps_mlp.tile([64, CG], F32, tag="mlp", space="PSUM")
                nc.tensor.matmul(prT_ps[:], lhsT=Wt('WoA'), rhs=eo[:, 0, :],
                                 start=True, stop=False)
                nc.tensor.matmul(prT_ps[:], lhsT=Wt('WoB'), rhs=eo[:, 1, :],
                                 start=False, stop=True)
                prT = gp.tile([64, CG], BF, tag="prT")
                nc.scalar.copy(prT[:], prT_ps[:])
                pre_ps = ps_big.tile([128, SPG * 64], F32, tag="big", space="PSUM")
                for j in range(SPG):
                    nc.tensor.matmul(pre_ps[:, j * 64:(j + 1) * 64],
                                     lhsT=prT[:, j * P:(j + 1) * P],
                                     rhs=Wt('I128', 64)[:, 0:64],
                                     start=True, stop=True)
                pre = gp.tile([128, SPG * 64], BF, tag="pre")
                nc.scalar.copy(pre[:], pre_ps[:])
                accg = ps_ln.tile([128, 64], F32, tag="lnbcab", space="PSUM")
                for j in range(SPG):
                    s = g * SPG + j
                    nc.tensor.matmul(accg[:], lhsT=ohw[:, s, :],
                                     rhs=pre[:, j * 64:(j + 1) * 64],
                                     start=(j == 0), stop=(j == SPG - 1))
                nc.vector.tensor_add(acc[:], acc[:], accg[:])

            nc.sync.dma_start(d_nout[w * 128:(w + 1) * 128, :], acc[:])

    nc.compile()
    return nc


# ------------------------------------------------------------------ driver
_GRAPH_CACHE = {}


def kernel(node_in, node_embed, edge_sh, edge_length_embedding,
           edge_src, edge_dst, batch, params):
    inputs = dict(node_in=node_in, node_embed=node_embed, edge_sh=edge_sh,
                  edge_length_embedding=edge_length_embedding,
                  edge_src=edge_src, edge_dst=edge_dst, batch=batch,
                  params=params)
    cores, meta = preprocess(inputs)
    W_max, NSRC = meta['W_max'], meta['NSRC']
    wpack = pack_weights(params)
    bpack = pack_biases(params)

    NWIN = int(os.environ.get('KNWIN', W_max))
    key = (W_max, NSRC, NWIN)
    if key not in _GRAPH_CACHE:
        _GRAPH_CACHE[key] = build_graph(W_max, NSRC, NWIN, wpack, bpack)
    nc = _GRAPH_CACHE[key]

    in_maps = []
    for c in cores:
        in_maps.append({
            "wei": wpack[0], "bia": bpack[0],
            "elenT": c['elenT'], "shT": c['shT'],
            "oheop": c['oheop'], "ohnop": c['ohnop'],
            "srcidx": c['srcidx'], "tsrc": c['table_src'],
            "dstfeat": c['dstfeat'],
        })
    trace = os.environ.get('KTRACE', '0') == '1'
    res = run_bass_kernel_spmd(nc, in_maps, core_ids=list(range(NCORES)),
                               trace=trace)
    if trace:
        print(f"HW exec time: {res.exec_time_ns} ns")

    node_out = np.zeros((N, DOUT), np.float32)
    esc_out = np.zeros((E, 32), np.float32)
    for k, c in enumerate(cores):
        r = res.results[k]
        nreal = min(c['n1'], N) - c['n0']
        node_out[c['n0']:c['n0'] + nreal] = r['nodeout'][:nreal]
        v = c['valid']
        esc_out[c['eid'][v]] = r['esc'].T[v]
    return node_out, esc_out


# revision 8
# speedup vs baseline: 1.0089x; 1.0089x over previous
"""Trainium2 Bass kernel for nn_Block_71554155151896 (GNN message passing).

Sharding: edges sorted by dst on host, split into 8 contiguous 128-aligned
dst-node ranges (one per core). All edges of a node live on one core, so
segment softmax + scatter-sum are core-local (no collectives). The device
does all model math (MLPs, LN, softmax, tensor product, gathers, one-hot
matmul scatter); the host only does index prep / padding / permutation.

Layout: feature-on-partition (fep) [d, edges] for the matmul chain, bf16
matmul inputs with f32 PSUM accumulation. Edges processed per 128-node
window (NWSUB=9 sub-chunks of 128 edges, G=3 matmul groups of 384).
"""
import sys
sys.path.insert(0, "/opt/trn_rl_repo")
import os
import numpy as np
import ml_dtypes
from contextlib import ExitStack

import concourse.bass as bass
import concourse.tile as tile
from concourse import bacc, mybir
from concourse.bass_utils import run_bass_kernel_spmd

bf16 = ml_dtypes.bfloat16
F32 = mybir.dt.float32
BF = mybir.dt.bfloat16
I32 = mybir.dt.int32

N, E, D, S, H, L = 50000, 400000, 64, 9, 4, 64
DOUT = 64
NCORES = 8
P = 128
NWSUB = 9
WE = NWSUB * P        # 1152 edges / window
G = 3
CG = WE // G          # 384
SPG = NWSUB // G      # 3

AF = mybir.ActivationFunctionType
OP = mybir.AluOpType


def _to_bf(x):
    return np.asarray(x, np.float32).astype(bf16)


# ------------------------------------------------------------------ host prep
def preprocess(inputs):
    src = np.asarray(inputs['edge_src'])
    dst = np.asarray(inputs['edge_dst'])
    elen = np.asarray(inputs['edge_length_embedding'])
    sh = np.asarray(inputs['edge_sh'])
    node = np.asarray(inputs['node_in'])

    order = np.argsort(dst, kind='stable')
    dst_s = dst[order]

    bounds = [0]
    for k in range(1, NCORES):
        target = k * E // NCORES
        bounds.append(min((int(dst_s[target]) + 127) // 128 * 128, N))
    bounds.append(((N + 127) // 128) * 128)
    W_max = max((bounds[k + 1] - bounds[k] + 127) // 128 for k in range(NCORES))

    cores = []
    for k in range(NCORES):
        n0, n1 = bounds[k], bounds[k + 1]
        lo = np.searchsorted(dst_s, n0, 'left')
        hi = np.searchsorted(dst_s, min(n1, N), 'left') if n1 <= N else E
        eidx = order[lo:hi]
        dstk = dst[eidx] - n0
        W = (n1 - n0 + 127) // 128

        eid = np.full((W_max, WE), -1, np.int64)
        for w in range(W):
            ids = eidx[(dstk // 128) == w]
            assert len(ids) <= WE, f"window overflow {len(ids)}"
            eid[w, :len(ids)] = ids
        eid = eid.reshape(-1)
        valid = eid >= 0
        eid_c = np.where(valid, eid, 0)
        Ep = eid.size

        uniq, src_rel = np.unique(src[eid_c], return_inverse=True)
        src_rel = np.where(valid, src_rel, len(uniq)).astype(np.int32)
        dst_rel = np.where(valid, dst[eid_c] - n0, 0)
        rel_in_win = dst_rel % 128

        ar = np.arange(128)
        oh_eop = ((rel_in_win[:, None] == ar[None, :]) &
                  valid[:, None]).astype(np.float32)           # [Ep, 128]
        # nop per window: [W_max*128 nodes, WE]
        oh_nop = np.zeros((W_max * 128, WE), bf16)
        for w in range(W_max):
            blk = oh_eop[w * WE:(w + 1) * WE]                  # [WE, 128]
            oh_nop[w * 128:(w + 1) * 128, :] = _to_bf(blk.T)

        table_src = np.zeros((len(uniq) + 128, 64), bf16)
        table_src[:len(uniq)] = _to_bf(node[uniq])

        nreal = min(n1, N) - n0
        dstfeat = np.zeros((W_max * 128, D), bf16)
        dstfeat[:nreal] = _to_bf(node[n0:n0 + nreal])

        elen_T = np.zeros((D, Ep), bf16)
        elen_T[:, valid] = _to_bf(elen[eid_c]).T[:, valid]
        sh_T = np.zeros((S, Ep), bf16)
        sh_T[:, valid] = _to_bf(sh[eid_c]).T[:, valid]

        srcidx = np.zeros((W_max, 128, NWSUB), np.int32)
        sr = src_rel.reshape(W_max, NWSUB, 128)
        srcidx[:] = np.transpose(sr, (0, 2, 1))

        cores.append(dict(
            n0=n0, n1=n1, W=W, eid=eid, valid=valid,
            elenT=elen_T, shT=sh_T, oheop=_to_bf(oh_eop),
            ohnop=oh_nop, srcidx=srcidx,
            table_src=table_src, dstfeat=dstfeat,
        ))
    NSRC = max(c['table_src'].shape[0] for c in cores)
    for c in cores:
        t = c['table_src']
        if t.shape[0] < NSRC:
            c['table_src'] = np.vstack(
                [t, np.zeros((NSRC - t.shape[0], 64), bf16)])
    return cores, dict(W_max=W_max, NSRC=NSRC, bounds=bounds)


def pack_weights(params):
    cols = {}
    buf = []
    pos = 0

    def add(name, mat):
        nonlocal pos
        mat = np.asarray(mat, np.float32)
        m = np.zeros((128, mat.shape[1]), np.float32)
        m[:mat.shape[0]] = mat
        buf.append(m)
        cols[name] = (pos, mat.shape[1])
        pos += mat.shape[1]

    def lhsT_aug(Wm, bm=None):
        Wm = np.asarray(Wm, np.float32)
        rows = [Wm] + ([np.asarray(bm, np.float32)[None, :]] if bm is not None else [])
        Wb = np.vstack(rows)
        return np.concatenate([Wb, Wb.mean(1, keepdims=True)], axis=1)

    p = params
    a, sc, tp2 = p['alpha'], p['scalar'], p['tp2']
    Wp = np.asarray(p['W_pre'])
    add('Wsrc2', np.concatenate([Wp[0:64]] * 2, axis=1))       # [64, 128]
    add('Wdst2', np.concatenate([Wp[64:128]] * 2, axis=1))
    add('Welen2', np.concatenate([Wp[128:192]] * 2, axis=1))
    add('Wa1', lhsT_aug(a['W1'], a['b1']))                     # [65, 65]
    add('Wa2', lhsT_aug(a['W2'], a['b2']))
    add('Wa3', np.vstack([np.asarray(a['W3']),
                          np.asarray(a['b3'])[None, :]]))      # [65, 4]
    add('Wr1', np.asarray(tp2['W_r1']))
    add('Wr2', np.asarray(tp2['W_r2']))                        # [64, 256]
    Wtp = np.asarray(tp2['W_tp'])
    Wflat = np.transpose(Wtp, (1, 0, 2)).reshape(S * D, 256)
    Wflat_p = np.zeros((5 * 128, 256), np.float32)
    Wflat_p[:S * D] = Wflat
    for t in range(5):
        add(f'Wtp{t}', Wflat_p[t * 128:(t + 1) * 128])
    We = np.asarray(p['W_edge'])
    add('WeA', We[0:128]); add('WeB', We[128:256])
    add('Ws1', lhsT_aug(sc['W1'], sc['b1']))
    add('Ws2', lhsT_aug(sc['W2'], sc['b2']))
    add('Ws3', np.vstack([np.asarray(sc['W3']),
                          np.asarray(sc['b3'])[None, :]]))     # [65, 32]
    Wo = np.asarray(p['W_out'])
    add('WoA', Wo[0:128]); add('WoB', Wo[128:256])

    add('ones64', np.full((64, 1), 1.0 / 64.0, np.float32))
    gpatA = np.zeros((1, 128), np.float32); gpatA[0, 0:64] = 1.0
    gpatB = np.zeros((1, 128), np.float32); gpatB[0, 64:128] = 1.0
    add('gpatA', gpatA); add('gpatB', gpatB)
    e4 = np.zeros((4, 256), np.float32)
    for h in range(4):
        e4[h, h * 64:(h + 1) * 64] = 1.0
    add('E4a', e4[:, 0:128]); add('E4b', e4[:, 128:256])
    add('I4', np.eye(4, dtype=np.float32))
    add('I128', np.eye(128, dtype=np.float32))
    for t in range(5):
        sel = np.zeros((S, 128), np.float32)
        for half in range(2):
            srow = 2 * t + half
            if srow < S:
                sel[srow, half * 64:(half + 1) * 64] = 1.0
        add(f'SEL{t}', sel)
    return _to_bf(np.concatenate(buf, axis=1)), cols


def pack_biases(params):
    cols = {}
    buf = []
    tp2 = params['tp2']
    for name, vec in [('br1', tp2['b_r1']),
                      ('br2a', np.asarray(tp2['b_r2'])[0:128]),
                      ('br2b', np.asarray(tp2['b_r2'])[128:256]),
                      ('zero', np.zeros(128))]:
        v = np.zeros((128, 1), np.float32)
        vv = np.asarray(vec, np.float32).ravel()
        v[:len(vv), 0] = vv
        cols[name] = len(buf)
        buf.append(v)
    return np.concatenate(buf, axis=1), cols


# ------------------------------------------------------------------ graph
def build_graph(W_max, NSRC, NWIN, wpack, bpack):
    Wpk, wc = wpack
    bpk, bc = bpack
    Ep = W_max * WE
    nc = bacc.Bacc('TRN2', target_bir_lowering=False, debug=False,
                   num_devices=NCORES)

    d_wei = nc.declare_dram_parameter("wei", list(Wpk.shape), BF, isOutput=False)
    d_bia = nc.declare_dram_parameter("bia", list(bpk.shape), F32, isOutput=False)
    d_elen = nc.declare_dram_parameter("elenT", [D, Ep], BF, isOutput=False)
    d_sh = nc.declare_dram_parameter("shT", [S, Ep], BF, isOutput=False)
    d_oh = nc.declare_dram_parameter("oheop", [Ep, 128], BF, isOutput=False)
    d_ohn = nc.declare_dram_parameter("ohnop", [W_max * 128, WE], BF, isOutput=False)
    d_sidx = nc.declare_dram_parameter("srcidx", [W_max, 128, NWSUB], I32, isOutput=False)
    d_tsrc = nc.declare_dram_parameter("tsrc", [NSRC, 64], BF, isOutput=False)
    d_dstf = nc.declare_dram_parameter("dstfeat", [W_max * 128, D], BF, isOutput=False)

    d_esc = nc.declare_dram_parameter("esc", [32, Ep], F32, isOutput=True)
    d_nout = nc.declare_dram_parameter("nodeout", [W_max * 128, DOUT], F32, isOutput=True)

    def raw_act(out, in_, func, bias, scale=1.0):
        eng = nc.scalar
        inputs = [eng.lower_ap(in_), eng.lower_ap(bias),
                  mybir.ImmediateValue(dtype=F32, value=float(scale)),
                  mybir.ImmediateValue(dtype=F32, value=0.0)]
        return eng.add_instruction(mybir.InstActivation(
            name=nc.get_next_instruction_name(),
            func=func, ins=inputs, outs=[eng.lower_ap(out)]))

    with ExitStack() as ctx:
        tc = ctx.enter_context(tile.TileContext(nc))
        cp = ctx.enter_context(tc.tile_pool(name="const", bufs=1))
        wp = ctx.enter_context(tc.tile_pool(name="win", bufs=2))
        gp = ctx.enter_context(tc.tile_pool(name="grp", bufs=2))
        ps_mlp = ctx.enter_context(tc.tile_pool(name="psm", bufs=2, space="PSUM"))
        ps_ln = ctx.enter_context(tc.tile_pool(name="psl", bufs=2, space="PSUM"))
        ps_ln1 = ctx.enter_context(tc.tile_pool(name="psl1", bufs=1, space="PSUM"))
        ps_tp = ctx.enter_context(tc.tile_pool(name="pst", bufs=2, space="PSUM"))
        ps_big = ctx.enter_context(tc.tile_pool(name="psb", bufs=1, space="PSUM"))

        wei = cp.tile([128, Wpk.shape[1]], BF, tag="wei")
        nc.sync.dma_start(wei[:], d_wei[:])
        bia = cp.tile([128, bpk.shape[1]], F32, tag="bia")
        nc.sync.dma_start(bia[:], d_bia[:])

        def Wt(nm, rows=128):
            o, n = wc[nm]
            return wei[0:rows, o:o + n]

        def Bi(nm, rows=128):
            return bia[0:rows, bc[nm]:bc[nm] + 1]

        nwin = cp.tile([128, W_max, 68], BF, tag="nwin")
        nc.sync.dma_start(
            nwin[:, :, 0:64],
            d_dstf[:].rearrange("(w p) d -> p w d", p=128))

        def ln_silu(x_ps, mlp_tag):
            """x_ps PSUM [65(+), CG]: rows 0:64 = x (incl bias), row 64 = mean.
            Returns SBUF bf16 [65, CG]: rows 0:64 silu(LN(x)), row 64 ones."""
            sq = gp.tile([64, CG], BF, tag="lnsq")
            nc.scalar.activation(sq[:], x_ps[0:64, :], AF.Square)
            msq = ps_ln1.tile([1, CG], F32, tag="lnmsq", space="PSUM")
            nc.tensor.matmul(msq[:], lhsT=Wt('ones64', 64), rhs=sq[:],
                             start=True, stop=True)
            musq = gp.tile([1, CG], F32, tag="lnmusq")
            nc.scalar.activation(musq[:], x_ps[64:65, :], AF.Square)
            var = gp.tile([1, CG], F32, tag="lnvar")
            nc.vector.scalar_tensor_tensor(out=var[:], in0=msq[:], scalar=1e-6,
                                           in1=musq[:], op0=OP.add,
                                           op1=OP.subtract)
            rstd = gp.tile([1, CG], BF, tag="lnrstd")
            raw_act(rstd[:], var[:], AF.Abs_reciprocal_sqrt,
                    bias=Bi('zero', 1))
            murstd = gp.tile([1, CG], BF, tag="lnmurstd")
            nc.vector.tensor_tensor(out=murstd[:], in0=x_ps[64:65, :],
                                    in1=rstd[:], op=OP.mult)
            bcab = ps_ln.tile([128, CG], F32, tag="lnbcab", space="PSUM")
            nc.tensor.matmul(bcab[:], lhsT=Wt('gpatA', 1), rhs=rstd[:],
                             start=True, stop=False)
            nc.tensor.matmul(bcab[:], lhsT=Wt('gpatB', 1), rhs=murstd[:],
                             start=False, stop=True)
            bcsa = gp.tile([64, CG], BF, tag="lnbcsa")
            nc.vector.tensor_copy(bcsa[:], bcab[0:64, :])
            bcsb = gp.tile([64, CG], BF, tag="lnbcsb")
            nc.scalar.copy(bcsb[:], bcab[64:128, :])
            t1 = gp.tile([64, CG], BF, tag="lnt1")
            nc.vector.tensor_tensor(out=t1[:], in0=x_ps[0:64, :],
                                    in1=bcsa[:], op=OP.mult)
            t2 = gp.tile([64, CG], BF, tag="lnt2")
            nc.vector.tensor_tensor(out=t2[:], in0=t1[:], in1=bcsb[:],
                                    op=OP.subtract)
            out = gp.tile([65, CG], BF, tag="ln_" + mlp_tag)
            nc.vector.memset(out[64:65, :], 1.0)
            nc.scalar.activation(out[0:64, :], t2[:], AF.Silu)
            return out

        for w in range(NWIN):
            e0 = w * WE
            elw = wp.tile([65, WE], BF, tag="elw")
            nc.sync.dma_start(elw[0:64, :], d_elen[:, e0:e0 + WE])
            nc.vector.memset(elw[64:65, :], 1.0)
            shw = wp.tile([S, WE], BF, tag="shw")
            nc.sync.dma_start(shw[:], d_sh[:, e0:e0 + WE])
            ohw = wp.tile([128, NWSUB, 128], BF, tag="ohw")
            nc.sync.dma_start(
                ohw[:], d_oh[e0:e0 + WE, :].rearrange("(s p) n -> p s n", p=128))
            ohn = wp.tile([128, WE], BF, tag="ohn")
            nc.sync.dma_start(ohn[:], d_ohn[w * 128:(w + 1) * 128, :])

            # ---- pass A: alpha logits -> ex -> den ----
            exw = wp.tile([4, WE], BF, tag="exw")
            for g in range(G):
                c0 = g * CG
                x1 = ps_mlp.tile([65, CG], F32, tag="mlp", space="PSUM")
                nc.tensor.matmul(x1[:], lhsT=Wt('Wa1', 65),
                                 rhs=elw[:, c0:c0 + CG], start=True, stop=True)
                h1 = ln_silu(x1, "a1")
                x2 = ps_mlp.tile([65, CG], F32, tag="mlp", space="PSUM")
                nc.tensor.matmul(x2[:], lhsT=Wt('Wa2', 65), rhs=h1[:],
                                 start=True, stop=True)
                h2 = ln_silu(x2, "a2")
                lg = ps_mlp.tile([4, CG], F32, tag="mlp", space="PSUM")
                nc.tensor.matmul(lg[:], lhsT=Wt('Wa3', 65), rhs=h2[:],
                                 start=True, stop=True)
                nc.scalar.activation(exw[:, c0:c0 + CG], lg[:], AF.Exp)

            exe_ps = ps_big.tile([128, NWSUB * 4], F32, tag="big", space="PSUM")
            for s in range(NWSUB):
                nc.tensor.matmul(exe_ps[:, s * 4:(s + 1) * 4],
                                 lhsT=exw[:, s * P:(s + 1) * P],
                                 rhs=Wt('I4', 4), start=True, stop=True)
            exe = wp.tile([128, NWSUB * 4], BF, tag="exe")
            nc.scalar.copy(exe[:], exe_ps[:])

            den_ps = ps_ln1.tile([128, 4], F32, tag="lnmsq", space="PSUM")
            for s in range(NWSUB):
                nc.tensor.matmul(den_ps[:], lhsT=ohw[:, s, :],
                                 rhs=exe[:, s * 4:(s + 1) * 4],
                                 start=(s == 0), stop=(s == NWSUB - 1))
            dent = wp.tile([128, 4], F32, tag="dent")
            nc.vector.tensor_scalar(out=dent[:], in0=den_ps[:], scalar1=1e-12,
                                    scalar2=None, op0=OP.add)
            dent2 = wp.tile([128, 4], F32, tag="dent2")
            nc.vector.reciprocal(out=dent2[:], in_=dent[:])
            nc.scalar.copy(nwin[:, w, 64:68], dent2[:])

            # ---- src gather ----
            srcw = wp.tile([128, NWSUB, 64], BF, tag="srcw")
            sidx = wp.tile([128, NWSUB], I32, tag="sidx")
            nc.sync.dma_start(sidx[:], d_sidx[w])
            for s in range(NWSUB):
                nc.gpsimd.indirect_dma_start(
                    out=srcw[:, s, :], out_offset=None, in_=d_tsrc[:],
                    in_offset=bass.IndirectOffsetOnAxis(
                        ap=sidx[:, s:s + 1], axis=0))

            # ---- per-group value chain ----
            acc = wp.tile([128, DOUT], F32, tag="acc")
            nc.vector.memset(acc[:], 0.0)

            for g in range(G):
                c0 = g * CG
                # srcT fep
                srcT_ps = ps_ln.tile([64, CG], F32, tag="lnbcab", space="PSUM")
                for j in range(SPG):
                    s = g * SPG + j
                    nc.tensor.matmul(srcT_ps[:, j * P:(j + 1) * P],
                                     lhsT=srcw[:, s, :], rhs=Wt('I128'),
                                     start=True, stop=True)
                srcT = gp.tile([64, CG], BF, tag="srcT")
                nc.vector.tensor_copy(srcT[:], srcT_ps[:])

                # dst features + recip den per edge (one MM per group)
                dstf_ps = ps_ln.tile([64, CG], F32, tag="lnbcab", space="PSUM")
                nc.tensor.matmul(dstf_ps[:], lhsT=nwin[:, w, 0:64],
                                 rhs=ohn[:, c0:c0 + CG], start=True, stop=True)
                dstT = gp.tile([64, CG], BF, tag="dstT")
                nc.vector.tensor_copy(dstT[:], dstf_ps[:])
                dden_ps = ps_mlp.tile([4, CG], F32, tag="mlp", space="PSUM")
                nc.tensor.matmul(dden_ps[:], lhsT=nwin[:, w, 64:68],
                                 rhs=ohn[:, c0:c0 + CG], start=True, stop=True)
                dden = gp.tile([4, CG], BF, tag="dden")
                nc.scalar.copy(dden[:], dden_ps[:])

                # msg (stacked x2)
                msg_ps = ps_big.tile([128, CG], F32, tag="big", space="PSUM")
                nc.tensor.matmul(msg_ps[:], lhsT=Wt('Wsrc2', 64), rhs=srcT[:],
                                 start=True, stop=False)
                nc.tensor.matmul(msg_ps[:], lhsT=Wt('Wdst2', 64),
                                 rhs=dstT[:], start=False, stop=False)
                nc.tensor.matmul(msg_ps[:], lhsT=Wt('Welen2', 64),
                                 rhs=elw[0:64, c0:c0 + CG], start=False, stop=True)
                msg2 = gp.tile([128, CG], BF, tag="msg2")
                nc.scalar.copy(msg2[:], msg_ps[:])

                # alpha = ex * recip_den[dst]
                alpha4 = gp.tile([4, CG], BF, tag="alpha4")
                nc.vector.tensor_tensor(out=alpha4[:], in0=exw[:, c0:c0 + CG],
                                        in1=dden[:], op=OP.mult)
                aexp_ps = ps_tp.tile([128, CG], F32, tag="tp", space="PSUM")
                nc.tensor.matmul(aexp_ps[:], lhsT=Wt('E4a', 4), rhs=alpha4[:],
                                 start=True, stop=True)
                aexp_ps2 = ps_tp.tile([128, CG], F32, tag="tp", space="PSUM")
                nc.tensor.matmul(aexp_ps2[:], lhsT=Wt('E4b', 4), rhs=alpha4[:],
                                 start=True, stop=True)
                aexp = gp.tile([128, 2, CG], BF, tag="aexp")
                nc.scalar.copy(aexp[:, 0, :], aexp_ps[:])
                nc.scalar.copy(aexp[:, 1, :], aexp_ps2[:])

                # radial, radalpha = (radial + br2) * alphaexp
                r1_ps = ps_mlp.tile([64, CG], F32, tag="mlp", space="PSUM")
                nc.tensor.matmul(r1_ps[:], lhsT=Wt('Wr1', 64),
                                 rhs=elw[0:64, c0:c0 + CG], start=True, stop=True)
                r1 = gp.tile([64, CG], BF, tag="r1")
                nc.scalar.activation(r1[:], r1_ps[:], AF.Silu, bias=Bi('br1', 64))
                radal = gp.tile([128, 2, CG], BF, tag="radal")
                for half in range(2):
                    rad_ps = ps_big.tile([128, CG], F32, tag="big", space="PSUM")
                    nc.tensor.matmul(
                        rad_ps[:], lhsT=Wt('Wr2', 64)[:, half * 128:(half + 1) * 128],
                        rhs=r1[:], start=True, stop=True)
                    nc.vector.scalar_tensor_tensor(
                        out=radal[:, half, :], in0=rad_ps[:],
                        scalar=Bi('br2a' if half == 0 else 'br2b'),
                        in1=aexp[:, half, :], op0=OP.add, op1=OP.mult)

                # kron + tensor product
                tpA = ps_tp.tile([128, CG], F32, tag="tp", space="PSUM")
                tpB = ps_tp.tile([128, CG], F32, tag="tp", space="PSUM")
                for t in range(5):
                    shp_ps = ps_big.tile([128, CG], F32, tag="big", space="PSUM")
                    nc.tensor.matmul(shp_ps[:], lhsT=Wt(f'SEL{t}', S),
                                     rhs=shw[:, c0:c0 + CG], start=True, stop=True)
                    shp = gp.tile([128, CG], BF, tag="shp")
                    nc.vector.tensor_copy(shp[:], shp_ps[:])
                    kro = gp.tile([128, CG], BF, tag="kro")
                    nc.vector.tensor_tensor(out=kro[:], in0=msg2[:], in1=shp[:],
                                            op=OP.mult)
                    nc.tensor.matmul(tpA[:], lhsT=Wt(f'Wtp{t}')[:, 0:128],
                                     rhs=kro[:], start=(t == 0), stop=(t == 4))
                    nc.tensor.matmul(tpB[:], lhsT=Wt(f'Wtp{t}')[:, 128:256],
                                     rhs=kro[:], start=(t == 0), stop=(t == 4))

                eo = gp.tile([128, 2, CG], BF, tag="eo")
                nc.vector.tensor_tensor(out=eo[:, 0, :], in0=tpA[:],
                                        in1=radal[:, 0, :], op=OP.mult)
                nc.vector.tensor_tensor(out=eo[:, 1, :], in0=tpB[:],
                                        in1=radal[:, 1, :], op=OP.mult)

                # edge scalar head
                xes_ps = ps_mlp.tile([64, CG], F32, tag="mlp", space="PSUM")
                nc.tensor.matmul(xes_ps[:], lhsT=Wt('WeA'), rhs=eo[:, 0, :],
                                 start=True, stop=False)
                nc.tensor.matmul(xes_ps[:], lhsT=Wt('WeB'), rhs=eo[:, 1, :],
                                 start=False, stop=True)
                xes = gp.tile([65, CG], BF, tag="xes")
                nc.vector.memset(xes[64:65, :], 1.0)
                nc.scalar.copy(xes[0:64, :], xes_ps[:])
                xs1 = ps_mlp.tile([65, CG], F32, tag="mlp", space="PSUM")
                nc.tensor.matmul(xs1[:], lhsT=Wt('Ws1', 65), rhs=xes[:],
                                 start=True, stop=True)
                hs1 = ln_silu(xs1, "s1")
                xs2 = ps_mlp.tile([65, CG], F32, tag="mlp", space="PSUM")
                nc.tensor.matmul(xs2[:], lhsT=Wt('Ws2', 65), rhs=hs1[:],
                                 start=True, stop=True)
                hs2 = ln_silu(xs2, "s2")
                esc_ps = ps_mlp.tile([32, CG], F32, tag="mlp", space="PSUM")
                nc.tensor.matmul(esc_ps[:], lhsT=Wt('Ws3', 65), rhs=hs2[:],
                                 start=True, stop=True)
                escs = gp.tile([32, CG], F32, tag="escs")
                nc.scalar.copy(escs[:], esc_ps[:])
                nc.sync.dma_start(d_esc[:, e0 + c0:e0 + c0 + CG], escs[:])

                # proj + scatter
                prT_ps = ps_mlp.tile([64, CG], F32, tag="mlp", space="PSUM")
                nc.tensor.matmul(prT_ps[:], lhsT=Wt('WoA'), rhs=eo[:, 0, :],
                                 start=True, stop=False)
                nc.tensor.matmul(prT_ps[:], lhsT=Wt('WoB'), rhs=eo[:, 1, :],
                                 start=False, stop=True)
                prT = gp.tile([64, CG], BF, tag="prT")
                nc.scalar.copy(prT[:], prT_ps[:])
                pre_ps = ps_big.tile([128, SPG * 64], F32, tag="big", space="PSUM")
                for j in range(SPG):
                    nc.tensor.matmul(pre_ps[:, j * 64:(j + 1) * 64],
                                     lhsT=prT[:, j * P:(j + 1) * P],
                                     rhs=Wt('I128', 64)[:, 0:64],
                                     start=True, stop=True)
                pre = gp.tile([128, SPG * 64], BF, tag="pre")
                nc.vector.tensor_copy(pre[:], pre_ps[:])
                accg = ps_ln.tile([128, 64], F32, tag="lnbcab", space="PSUM")
                for j in range(SPG):
                    s = g * SPG + j
                    nc.tensor.matmul(accg[:], lhsT=ohw[:, s, :],
                                     rhs=pre[:, j * 64:(j + 1) * 64],
                                     start=(j == 0), stop=(j == SPG - 1))
                nc.vector.tensor_add(acc[:], acc[:], accg[:])

            nc.sync.dma_start(d_nout[w * 128:(w + 1) * 128, :], acc[:])

    nc.compile()
    return nc


# ------------------------------------------------------------------ driver
_GRAPH_CACHE = {}


def kernel(node_in, node_embed, edge_sh, edge_length_embedding,
           edge_src, edge_dst, batch, params):
    inputs = dict(node_in=node_in, node_embed=node_embed, edge_sh=edge_sh,
                  edge_length_embedding=edge_length_embedding,
                  edge_src=edge_src, edge_dst=edge_dst, batch=batch,
                  params=params)
    cores, meta = preprocess(inputs)
    W_max, NSRC = meta['W_max'], meta['NSRC']
    wpack = pack_weights(params)
    bpack = pack_biases(params)

    NWIN = int(os.environ.get('KNWIN', W_max))
    key = (W_max, NSRC, NWIN)
    if key not in _GRAPH_CACHE:
        _GRAPH_CACHE[key] = build_graph(W_max, NSRC, NWIN, wpack, bpack)
    nc = _GRAPH_CACHE[key]

    in_maps = []
    for c in cores:
        in_maps.append({
            "wei": wpack[0], "bia": bpack[0],
            "elenT": c['elenT'], "shT": c['shT'],
            "oheop": c['oheop'], "ohnop": c['ohnop'],
            "srcidx": c['srcidx'], "tsrc": c['table_src'],
            "dstfeat": c['dstfeat'],
        })
    trace = os.environ.get('KTRACE', '0') == '1'
    res = run_bass_kernel_spmd(nc, in_maps, core_ids=list(range(NCORES)),
                               trace=trace)
    if trace:
        print(f"HW exec time: {res.exec_time_ns} ns")

    node_out = np.zeros((N, DOUT), np.float32)
    esc_out = np.zeros((E, 32), np.float32)
    for k, c in enumerate(cores):
        r = res.results[k]
        nreal = min(c['n1'], N) - c['n0']
        node_out[c['n0']:c['n0'] + nreal] = r['nodeout'][:nreal]
        v = c['valid']
        esc_out[c['eid'][v]] = r['esc'].T[v]
    return node_out, esc_out


# revision 9
# speedup vs baseline: 1.0091x; 1.0002x over previous
"""Trainium2 Bass kernel for nn_Block_71554155151896 (GNN message passing).

Sharding: edges sorted by dst on host, split into 8 contiguous 128-aligned
dst-node ranges (one per core). All edges of a node live on one core, so
segment softmax + scatter-sum are core-local (no collectives). The device
does all model math (MLPs, LN, softmax, tensor product, gathers, one-hot
matmul scatter); the host only does index prep / padding / permutation.

Layout: feature-on-partition (fep) [d, edges] for the matmul chain, bf16
matmul inputs with f32 PSUM accumulation. Edges processed per 128-node
window (NWSUB=9 sub-chunks of 128 edges, G=3 matmul groups of 384).
"""
import sys
sys.path.insert(0, "/opt/trn_rl_repo")
import os
import numpy as np
import ml_dtypes
from contextlib import ExitStack

import concourse.bass as bass
import concourse.tile as tile
from concourse import bacc, mybir
from concourse.bass_utils import run_bass_kernel_spmd

bf16 = ml_dtypes.bfloat16
F32 = mybir.dt.float32
BF = mybir.dt.bfloat16
I32 = mybir.dt.int32

N, E, D, S, H, L = 50000, 400000, 64, 9, 4, 64
DOUT = 64
NCORES = 8
P = 128
NWSUB = 9
WE = NWSUB * P        # 1152 edges / window
G = 3
CG = WE // G          # 384
SPG = NWSUB // G      # 3

AF = mybir.ActivationFunctionType
OP = mybir.AluOpType


def _to_bf(x):
    return np.asarray(x, np.float32).astype(bf16)


# ------------------------------------------------------------------ host prep
def preprocess(inputs):
    src = np.asarray(inputs['edge_src'])
    dst = np.asarray(inputs['edge_dst'])
    elen = np.asarray(inputs['edge_length_embedding'])
    sh = np.asarray(inputs['edge_sh'])
    node = np.asarray(inputs['node_in'])

    order = np.argsort(dst, kind='stable')
    dst_s = dst[order]

    bounds = [0]
    for k in range(1, NCORES):
        target = k * E // NCORES
        bounds.append(min((int(dst_s[target]) + 127) // 128 * 128, N))
    bounds.append(((N + 127) // 128) * 128)
    W_max = max((bounds[k + 1] - bounds[k] + 127) // 128 for k in range(NCORES))

    cores = []
    for k in range(NCORES):
        n0, n1 = bounds[k], bounds[k + 1]
        lo = np.searchsorted(dst_s, n0, 'left')
        hi = np.searchsorted(dst_s, min(n1, N), 'left') if n1 <= N else E
        eidx = order[lo:hi]
        dstk = dst[eidx] - n0
        W = (n1 - n0 + 127) // 128

        eid = np.full((W_max, WE), -1, np.int64)
        for w in range(W):
            ids = eidx[(dstk // 128) == w]
            assert len(ids) <= WE, f"window overflow {len(ids)}"
            eid[w, :len(ids)] = ids
        eid = eid.reshape(-1)
        valid = eid >= 0
        eid_c = np.where(valid, eid, 0)
        Ep = eid.size

        uniq, src_rel = np.unique(src[eid_c], return_inverse=True)
        src_rel = np.where(valid, src_rel, len(uniq)).astype(np.int32)
        dst_rel = np.where(valid, dst[eid_c] - n0, 0)
        rel_in_win = dst_rel % 128

        ar = np.arange(128)
        oh_eop = ((rel_in_win[:, None] == ar[None, :]) &
                  valid[:, None]).astype(np.float32)           # [Ep, 128]
        # nop per window: [W_max*128 nodes, WE]
        oh_nop = np.zeros((W_max * 128, WE), bf16)
        for w in range(W_max):
            blk = oh_eop[w * WE:(w + 1) * WE]                  # [WE, 128]
            oh_nop[w * 128:(w + 1) * 128, :] = _to_bf(blk.T)

        table_src = np.zeros((len(uniq) + 128, 64), bf16)
        table_src[:len(uniq)] = _to_bf(node[uniq])

        nreal = min(n1, N) - n0
        dstfeat = np.zeros((W_max * 128, D), bf16)
        dstfeat[:nreal] = _to_bf(node[n0:n0 + nreal])

        elen_T = np.zeros((D, Ep), bf16)
        elen_T[:, valid] = _to_bf(elen[eid_c]).T[:, valid]
        sh_T = np.zeros((S, Ep), bf16)
        sh_T[:, valid] = _to_bf(sh[eid_c]).T[:, valid]

        srcidx = np.zeros((W_max, 128, NWSUB), np.int32)
        sr = src_rel.reshape(W_max, NWSUB, 128)
        srcidx[:] = np.transpose(sr, (0, 2, 1))

        cores.append(dict(
            n0=n0, n1=n1, W=W, eid=eid, valid=valid,
            elenT=elen_T, shT=sh_T, oheop=_to_bf(oh_eop),
            ohnop=oh_nop, srcidx=srcidx,
            table_src=table_src, dstfeat=dstfeat,
        ))
    NSRC = max(c['table_src'].shape[0] for c in cores)
    for c in cores:
        t = c['table_src']
        if t.shape[0] < NSRC:
            c['table_src'] = np.vstack(
                [t, np.zeros((NSRC - t.shape[0], 64), bf16)])
    return cores, dict(W_max=W_max, NSRC=NSRC, bounds=bounds)


def pack_weights(params):
    cols = {}
    buf = []
    pos = 0

    def add(name, mat):
        nonlocal pos
        mat = np.asarray(mat, np.float32)
        m = np.zeros((128, mat.shape[1]), np.float32)
        m[:mat.shape[0]] = mat
        buf.append(m)
        cols[name] = (pos, mat.shape[1])
        pos += mat.shape[1]

    def lhsT_aug(Wm, bm=None):
        Wm = np.asarray(Wm, np.float32)
        rows = [Wm] + ([np.asarray(bm, np.float32)[None, :]] if bm is not None else [])
        Wb = np.vstack(rows)
        return np.concatenate([Wb, Wb.mean(1, keepdims=True)], axis=1)

    p = params
    a, sc, tp2 = p['alpha'], p['scalar'], p['tp2']
    Wp = np.asarray(p['W_pre'])
    add('Wsrc2', np.concatenate([Wp[0:64]] * 2, axis=1))       # [64, 128]
    add('Wdst2', np.concatenate([Wp[64:128]] * 2, axis=1))
    add('Welen2', np.concatenate([Wp[128:192]] * 2, axis=1))
    add('Wa1', lhsT_aug(a['W1'], a['b1']))                     # [65, 65]
    add('Wa2', lhsT_aug(a['W2'], a['b2']))
    add('Wa3', np.vstack([np.asarray(a['W3']),
                          np.asarray(a['b3'])[None, :]]))      # [65, 4]
    add('Wr1', np.asarray(tp2['W_r1']))
    add('Wr2', np.asarray(tp2['W_r2']))                        # [64, 256]
    Wtp = np.asarray(tp2['W_tp'])
    Wflat = np.transpose(Wtp, (1, 0, 2)).reshape(S * D, 256)
    Wflat_p = np.zeros((5 * 128, 256), np.float32)
    Wflat_p[:S * D] = Wflat
    for t in range(5):
        add(f'Wtp{t}', Wflat_p[t * 128:(t + 1) * 128])
    We = np.asarray(p['W_edge'])
    add('WeA', We[0:128]); add('WeB', We[128:256])
    add('Ws1', lhsT_aug(sc['W1'], sc['b1']))
    add('Ws2', lhsT_aug(sc['W2'], sc['b2']))
    add('Ws3', np.vstack([np.asarray(sc['W3']),
                          np.asarray(sc['b3'])[None, :]]))     # [65, 32]
    Wo = np.asarray(p['W_out'])
    add('WoA', Wo[0:128]); add('WoB', Wo[128:256])

    add('ones64', np.full((64, 1), 1.0 / 64.0, np.float32))
    gpatA = np.zeros((1, 128), np.float32); gpatA[0, 0:64] = 1.0
    gpatB = np.zeros((1, 128), np.float32); gpatB[0, 64:128] = 1.0
    add('gpatA', gpatA); add('gpatB', gpatB)
    e4 = np.zeros((4, 256), np.float32)
    for h in range(4):
        e4[h, h * 64:(h + 1) * 64] = 1.0
    add('E4a', e4[:, 0:128]); add('E4b', e4[:, 128:256])
    add('I4', np.eye(4, dtype=np.float32))
    add('I128', np.eye(128, dtype=np.float32))
    for t in range(5):
        sel = np.zeros((S, 128), np.float32)
        for half in range(2):
            srow = 2 * t + half
            if srow < S:
                sel[srow, half * 64:(half + 1) * 64] = 1.0
        add(f'SEL{t}', sel)
    return _to_bf(np.concatenate(buf, axis=1)), cols


def pack_biases(params):
    cols = {}
    buf = []
    tp2 = params['tp2']
    for name, vec in [('br1', tp2['b_r1']),
                      ('br2a', np.asarray(tp2['b_r2'])[0:128]),
                      ('br2b', np.asarray(tp2['b_r2'])[128:256]),
                      ('zero', np.zeros(128))]:
        v = np.zeros((128, 1), np.float32)
        vv = np.asarray(vec, np.float32).ravel()
        v[:len(vv), 0] = vv
        cols[name] = len(buf)
        buf.append(v)
    return np.concatenate(buf, axis=1), cols


# ------------------------------------------------------------------ graph
def build_graph(W_max, NSRC, NWIN, wpack, bpack):
    Wpk, wc = wpack
    bpk, bc = bpack
    Ep = W_max * WE
    nc = bacc.Bacc('TRN2', target_bir_lowering=False, debug=False,
                   num_devices=NCORES)

    d_wei = nc.declare_dram_parameter("wei", list(Wpk.shape), BF, isOutput=False)
    d_bia = nc.declare_dram_parameter("bia", list(bpk.shape), F32, isOutput=False)
    d_elen = nc.declare_dram_parameter("elenT", [D, Ep], BF, isOutput=False)
    d_sh = nc.declare_dram_parameter("shT", [S, Ep], BF, isOutput=False)
    d_oh = nc.declare_dram_parameter("oheop", [Ep, 128], BF, isOutput=False)
    d_ohn = nc.declare_dram_parameter("ohnop", [W_max * 128, WE], BF, isOutput=False)
    d_sidx = nc.declare_dram_parameter("srcidx", [W_max, 128, NWSUB], I32, isOutput=False)
    d_tsrc = nc.declare_dram_parameter("tsrc", [NSRC, 64], BF, isOutput=False)
    d_dstf = nc.declare_dram_parameter("dstfeat", [W_max * 128, D], BF, isOutput=False)

    d_esc = nc.declare_dram_parameter("esc", [32, Ep], F32, isOutput=True)
    d_nout = nc.declare_dram_parameter("nodeout", [W_max * 128, DOUT], F32, isOutput=True)

    def raw_act(out, in_, func, bias, scale=1.0):
        eng = nc.scalar
        inputs = [eng.lower_ap(in_), eng.lower_ap(bias),
                  mybir.ImmediateValue(dtype=F32, value=float(scale)),
                  mybir.ImmediateValue(dtype=F32, value=0.0)]
        return eng.add_instruction(mybir.InstActivation(
            name=nc.get_next_instruction_name(),
            func=func, ins=inputs, outs=[eng.lower_ap(out)]))

    with ExitStack() as ctx:
        tc = ctx.enter_context(tile.TileContext(nc))
        cp = ctx.enter_context(tc.tile_pool(name="const", bufs=1))
        wp = ctx.enter_context(tc.tile_pool(name="win", bufs=3))
        gp = ctx.enter_context(tc.tile_pool(name="grp", bufs=4))
        ps_mlp = ctx.enter_context(tc.tile_pool(name="psm", bufs=2, space="PSUM"))
        ps_ln = ctx.enter_context(tc.tile_pool(name="psl", bufs=2, space="PSUM"))
        ps_ln1 = ctx.enter_context(tc.tile_pool(name="psl1", bufs=1, space="PSUM"))
        ps_tp = ctx.enter_context(tc.tile_pool(name="pst", bufs=2, space="PSUM"))
        ps_big = ctx.enter_context(tc.tile_pool(name="psb", bufs=1, space="PSUM"))

        wei = cp.tile([128, Wpk.shape[1]], BF, tag="wei")
        nc.sync.dma_start(wei[:], d_wei[:])
        bia = cp.tile([128, bpk.shape[1]], F32, tag="bia")
        nc.sync.dma_start(bia[:], d_bia[:])

        def Wt(nm, rows=128):
            o, n = wc[nm]
            return wei[0:rows, o:o + n]

        def Bi(nm, rows=128):
            return bia[0:rows, bc[nm]:bc[nm] + 1]

        nwin = cp.tile([128, W_max, 64], BF, tag="nwin")
        nc.sync.dma_start(
            nwin[:], d_dstf[:].rearrange("(w p) d -> p w d", p=128))

        def ln_silu(x_ps, mlp_tag):
            """x_ps PSUM [65(+), CG]: rows 0:64 = x (incl bias), row 64 = mean.
            Returns SBUF bf16 [65, CG]: rows 0:64 silu(LN(x)), row 64 ones."""
            sq = gp.tile([64, CG], BF, tag="lnsq")
            nc.scalar.activation(sq[:], x_ps[0:64, :], AF.Square)
            msq = ps_ln1.tile([1, CG], F32, tag="lnmsq", space="PSUM")
            nc.tensor.matmul(msq[:], lhsT=Wt('ones64', 64), rhs=sq[:],
                             start=True, stop=True)
            musq = gp.tile([1, CG], F32, tag="lnmusq")
            nc.scalar.activation(musq[:], x_ps[64:65, :], AF.Square)
            var = gp.tile([1, CG], F32, tag="lnvar")
            nc.vector.scalar_tensor_tensor(out=var[:], in0=msq[:], scalar=1e-6,
                                           in1=musq[:], op0=OP.add,
                                           op1=OP.subtract)
            rstd = gp.tile([1, CG], BF, tag="lnrstd")
            raw_act(rstd[:], var[:], AF.Abs_reciprocal_sqrt,
                    bias=Bi('zero', 1))
            murstd = gp.tile([1, CG], BF, tag="lnmurstd")
            nc.vector.tensor_tensor(out=murstd[:], in0=x_ps[64:65, :],
                                    in1=rstd[:], op=OP.mult)
            bcab = ps_ln.tile([128, CG], F32, tag="lnbcab", space="PSUM")
            nc.tensor.matmul(bcab[:], lhsT=Wt('gpatA', 1), rhs=rstd[:],
                             start=True, stop=False)
            nc.tensor.matmul(bcab[:], lhsT=Wt('gpatB', 1), rhs=murstd[:],
                             start=False, stop=True)
            bcsa = gp.tile([64, CG], BF, tag="lnbcsa")
            nc.vector.tensor_copy(bcsa[:], bcab[0:64, :])
            bcsb = gp.tile([64, CG], BF, tag="lnbcsb")
            nc.scalar.copy(bcsb[:], bcab[64:128, :])
            t1 = gp.tile([64, CG], BF, tag="lnt1")
            nc.vector.tensor_tensor(out=t1[:], in0=x_ps[0:64, :],
                                    in1=bcsa[:], op=OP.mult)
            t2 = gp.tile([64, CG], BF, tag="lnt2")
            nc.vector.tensor_tensor(out=t2[:], in0=t1[:], in1=bcsb[:],
                                    op=OP.subtract)
            out = gp.tile([65, CG], BF, tag="ln_" + mlp_tag)
            nc.vector.memset(out[64:65, :], 1.0)
            nc.scalar.activation(out[0:64, :], t2[:], AF.Silu)
            return out

        for w in range(NWIN):
            e0 = w * WE
            elw = wp.tile([65, WE], BF, tag="elw")
            nc.sync.dma_start(elw[0:64, :], d_elen[:, e0:e0 + WE])
            nc.vector.memset(elw[64:65, :], 1.0)
            shw = wp.tile([S, WE], BF, tag="shw")
            nc.sync.dma_start(shw[:], d_sh[:, e0:e0 + WE])
            ohw = wp.tile([128, NWSUB, 128], BF, tag="ohw")
            nc.sync.dma_start(
                ohw[:], d_oh[e0:e0 + WE, :].rearrange("(s p) n -> p s n", p=128))
            ohn = wp.tile([128, WE], BF, tag="ohn")
            nc.sync.dma_start(ohn[:], d_ohn[w * 128:(w + 1) * 128, :])

            # ---- pass A: alpha logits -> ex -> den ----
            exw = wp.tile([4, WE], BF, tag="exw")
            for g in range(G):
                c0 = g * CG
                x1 = ps_mlp.tile([65, CG], F32, tag="mlp", space="PSUM")
                nc.tensor.matmul(x1[:], lhsT=Wt('Wa1', 65),
                                 rhs=elw[:, c0:c0 + CG], start=True, stop=True)
                h1 = ln_silu(x1, "a1")
                x2 = ps_mlp.tile([65, CG], F32, tag="mlp", space="PSUM")
                nc.tensor.matmul(x2[:], lhsT=Wt('Wa2', 65), rhs=h1[:],
                                 start=True, stop=True)
                h2 = ln_silu(x2, "a2")
                lg = ps_mlp.tile([4, CG], F32, tag="mlp", space="PSUM")
                nc.tensor.matmul(lg[:], lhsT=Wt('Wa3', 65), rhs=h2[:],
                                 start=True, stop=True)
                nc.scalar.activation(exw[:, c0:c0 + CG], lg[:], AF.Exp)

            exe_ps = ps_big.tile([128, NWSUB * 4], F32, tag="big", space="PSUM")
            for s in range(NWSUB):
                nc.tensor.matmul(exe_ps[:, s * 4:(s + 1) * 4],
                                 lhsT=exw[:, s * P:(s + 1) * P],
                                 rhs=Wt('I4', 4), start=True, stop=True)
            exe = wp.tile([128, NWSUB * 4], BF, tag="exe")
            nc.scalar.copy(exe[:], exe_ps[:])

            den_ps = ps_ln1.tile([128, 4], F32, tag="lnmsq", space="PSUM")
            for s in range(NWSUB):
                nc.tensor.matmul(den_ps[:], lhsT=ohw[:, s, :],
                                 rhs=exe[:, s * 4:(s + 1) * 4],
                                 start=(s == 0), stop=(s == NWSUB - 1))
            dent = wp.tile([128, 4], F32, tag="dent")
            nc.vector.tensor_scalar(out=dent[:], in0=den_ps[:], scalar1=1e-12,
                                    scalar2=None, op0=OP.add)
            dent2 = wp.tile([128, 4], F32, tag="dent2")
            nc.vector.reciprocal(out=dent2[:], in_=dent[:])
            denw = wp.tile([128, 4], BF, tag="denw")
            nc.scalar.copy(denw[:], dent2[:])

            # ---- src gather ----
            srcw = wp.tile([128, NWSUB, 64], BF, tag="srcw")
            sidx = wp.tile([128, NWSUB], I32, tag="sidx")
            nc.sync.dma_start(sidx[:], d_sidx[w])
            for s in range(NWSUB):
                nc.gpsimd.indirect_dma_start(
                    out=srcw[:, s, :], out_offset=None, in_=d_tsrc[:],
                    in_offset=bass.IndirectOffsetOnAxis(
                        ap=sidx[:, s:s + 1], axis=0))

            # ---- per-group value chain ----
            acc = wp.tile([128, DOUT], F32, tag="acc")
            nc.vector.memset(acc[:], 0.0)

            for g in range(G):
                c0 = g * CG
                # srcT fep
                srcT_ps = ps_ln.tile([64, CG], F32, tag="lnbcab", space="PSUM")
                for j in range(SPG):
                    s = g * SPG + j
                    nc.tensor.matmul(srcT_ps[:, j * P:(j + 1) * P],
                                     lhsT=srcw[:, s, :], rhs=Wt('I128'),
                                     start=True, stop=True)
                srcT = gp.tile([64, CG], BF, tag="srcT")
                nc.vector.tensor_copy(srcT[:], srcT_ps[:])

                # dst features + recip den per edge (one MM per group)
                dstf_ps = ps_ln.tile([64, CG], F32, tag="lnbcab", space="PSUM")
                nc.tensor.matmul(dstf_ps[:], lhsT=nwin[:, w, :],
                                 rhs=ohn[:, c0:c0 + CG], start=True, stop=True)
                dstT = gp.tile([64, CG], BF, tag="dstT")
                nc.vector.tensor_copy(dstT[:], dstf_ps[:])
                dden_ps = ps_mlp.tile([4, CG], F32, tag="mlp", space="PSUM")
                nc.tensor.matmul(dden_ps[:], lhsT=denw[:],
                                 rhs=ohn[:, c0:c0 + CG], start=True, stop=True)
                dden = gp.tile([4, CG], BF, tag="dden")
                nc.scalar.copy(dden[:], dden_ps[:])

                # msg (stacked x2)
                msg_ps = ps_big.tile([128, CG], F32, tag="big", space="PSUM")
                nc.tensor.matmul(msg_ps[:], lhsT=Wt('Wsrc2', 64), rhs=srcT[:],
                                 start=True, stop=False)
                nc.tensor.matmul(msg_ps[:], lhsT=Wt('Wdst2', 64),
                                 rhs=dstT[:], start=False, stop=False)
                nc.tensor.matmul(msg_ps[:], lhsT=Wt('Welen2', 64),
                                 rhs=elw[0:64, c0:c0 + CG], start=False, stop=True)
                msg2 = gp.tile([128, CG], BF, tag="msg2")
                nc.scalar.copy(msg2[:], msg_ps[:])

                # alpha = ex * recip_den[dst]
                alpha4 = gp.tile([4, CG], BF, tag="alpha4")
                nc.vector.tensor_tensor(out=alpha4[:], in0=exw[:, c0:c0 + CG],
                                        in1=dden[:], op=OP.mult)
                aexp_ps = ps_tp.tile([128, CG], F32, tag="tp", space="PSUM")
                nc.tensor.matmul(aexp_ps[:], lhsT=Wt('E4a', 4), rhs=alpha4[:],
                                 start=True, stop=True)
                aexp_ps2 = ps_tp.tile([128, CG], F32, tag="tp", space="PSUM")
                nc.tensor.matmul(aexp_ps2[:], lhsT=Wt('E4b', 4), rhs=alpha4[:],
                                 start=True, stop=True)
                aexp = gp.tile([128, 2, CG], BF, tag="aexp")
                nc.scalar.copy(aexp[:, 0, :], aexp_ps[:])
                nc.scalar.copy(aexp[:, 1, :], aexp_ps2[:])

                # radial, radalpha = (radial + br2) * alphaexp
                r1_ps = ps_mlp.tile([64, CG], F32, tag="mlp", space="PSUM")
                nc.tensor.matmul(r1_ps[:], lhsT=Wt('Wr1', 64),
                                 rhs=elw[0:64, c0:c0 + CG], start=True, stop=True)
                r1 = gp.tile([64, CG], BF, tag="r1")
                nc.scalar.activation(r1[:], r1_ps[:], AF.Silu, bias=Bi('br1', 64))
                radal = gp.tile([128, 2, CG], BF, tag="radal")
                for half in range(2):
                    rad_ps = ps_big.tile([128, CG], F32, tag="big", space="PSUM")
                    nc.tensor.matmul(
                        rad_ps[:], lhsT=Wt('Wr2', 64)[:, half * 128:(half + 1) * 128],
                        rhs=r1[:], start=True, stop=True)
                    nc.vector.scalar_tensor_tensor(
                        out=radal[:, half, :], in0=rad_ps[:],
                        scalar=Bi('br2a' if half == 0 else 'br2b'),
                        in1=aexp[:, half, :], op0=OP.add, op1=OP.mult)

                # kron + tensor product
                tpA = ps_tp.tile([128, CG], F32, tag="tp", space="PSUM")
                tpB = ps_tp.tile([128, CG], F32, tag="tp", space="PSUM")
                for t in range(5):
                    shp_ps = ps_big.tile([128, CG], F32, tag="big", space="PSUM")
                    nc.tensor.matmul(shp_ps[:], lhsT=Wt(f'SEL{t}', S),
                                     rhs=shw[:, c0:c0 + CG], start=True, stop=True)
                    shp = gp.tile([128, CG], BF, tag="shp")
                    nc.vector.tensor_copy(shp[:], shp_ps[:])
                    kro = gp.tile([128, CG], BF, tag="kro")
                    nc.vector.tensor_tensor(out=kro[:], in0=msg2[:], in1=shp[:],
                                            op=OP.mult)
                    nc.tensor.matmul(tpA[:], lhsT=Wt(f'Wtp{t}')[:, 0:128],
                                     rhs=kro[:], start=(t == 0), stop=(t == 4))
                    nc.tensor.matmul(tpB[:], lhsT=Wt(f'Wtp{t}')[:, 128:256],
                                     rhs=kro[:], start=(t == 0), stop=(t == 4))

                eo = gp.tile([128, 2, CG], BF, tag="eo")
                nc.vector.tensor_tensor(out=eo[:, 0, :], in0=tpA[:],
                                        in1=radal[:, 0, :], op=OP.mult)
                nc.vector.tensor_tensor(out=eo[:, 1, :], in0=tpB[:],
                                        in1=radal[:, 1, :], op=OP.mult)

                # edge scalar head
                xes_ps = ps_mlp.tile([64, CG], F32, tag="mlp", space="PSUM")
                nc.tensor.matmul(xes_ps[:], lhsT=Wt('WeA'), rhs=eo[:, 0, :],
                                 start=True, stop=False)
                nc.tensor.matmul(xes_ps[:], lhsT=Wt('WeB'), rhs=eo[:, 1, :],
                                 start=False, stop=True)
                xes = gp.tile([65, CG], BF, tag="xes")
                nc.vector.memset(xes[64:65, :], 1.0)
                nc.scalar.copy(xes[0:64, :], xes_ps[:])
                xs1 = ps_mlp.tile([65, CG], F32, tag="mlp", space="PSUM")
                nc.tensor.matmul(xs1[:], lhsT=Wt('Ws1', 65), rhs=xes[:],
                                 start=True, stop=True)
                hs1 = ln_silu(xs1, "s1")
                xs2 = ps_mlp.tile([65, CG], F32, tag="mlp", space="PSUM")
                nc.tensor.matmul(xs2[:], lhsT=Wt('Ws2', 65), rhs=hs1[:],
                                 start=True, stop=True)
                hs2 = ln_silu(xs2, "s2")
                esc_ps = ps_mlp.tile([32, CG], F32, tag="mlp", space="PSUM")
                nc.tensor.matmul(esc_ps[:], lhsT=Wt('Ws3', 65), rhs=hs2[:],
                                 start=True, stop=True)
                escs = gp.tile([32, CG], F32, tag="escs")
                nc.scalar.copy(escs[:], esc_ps[:])
                nc.sync.dma_start(d_esc[:, e0 + c0:e0 + c0 + CG], escs[:])

                # proj + scatter
                prT_ps = ps_mlp.tile([64, CG], F32, tag="mlp", space="PSUM")
                nc.tensor.matmul(prT_ps[:], lhsT=Wt('WoA'), rhs=eo[:, 0, :],
                                 start=True, stop=False)
                nc.tensor.matmul(prT_ps[:], lhsT=Wt('WoB'), rhs=eo[:, 1, :],
                                 start=False, stop=True)
                prT = gp.tile([64, CG], BF, tag="prT")
                nc.scalar.copy(prT[:], prT_ps[:])
                pre_ps = ps_big.tile([128, SPG * 64], F32, tag="big", space="PSUM")
                for j in range(SPG):
                    nc.tensor.matmul(pre_ps[:, j * 64:(j + 1) * 64],
                                     lhsT=prT[:, j * P:(j + 1) * P],
                                     rhs=Wt('I128', 64)[:, 0:64],
                                     start=True, stop=True)
                pre = gp.tile([128, SPG * 64], BF, tag="pre")
                nc.vector.tensor_copy(pre[:], pre_ps[:])
                accg = ps_ln.tile([128, 64], F32, tag="lnbcab", space="PSUM")
                for j in range(SPG):
                    s = g * SPG + j
                    nc.tensor.matmul(accg[:], lhsT=ohw[:, s, :],
                                     rhs=pre[:, j * 64:(j + 1) * 64],
                                     start=(j == 0), stop=(j == SPG - 1))
                nc.vector.tensor_add(acc[:], acc[:], accg[:])

            nc.sync.dma_start(d_nout[w * 128:(w + 1) * 128, :], acc[:])

    nc.compile()
    return nc


# ------------------------------------------------------------------ driver
_GRAPH_CACHE = {}


def kernel(node_in, node_embed, edge_sh, edge_length_embedding,
           edge_src, edge_dst, batch, params):
    inputs = dict(node_in=node_in, node_embed=node_embed, edge_sh=edge_sh,
                  edge_length_embedding=edge_length_embedding,
                  edge_src=edge_src, edge_dst=edge_dst, batch=batch,
                  params=params)
    cores, meta = preprocess(inputs)
    W_max, NSRC = meta['W_max'], meta['NSRC']
    wpack = pack_weights(params)
    bpack = pack_biases(params)

    NWIN = int(os.environ.get('KNWIN', W_max))
    key = (W_max, NSRC, NWIN)
    if key not in _GRAPH_CACHE:
        _GRAPH_CACHE[key] = build_graph(W_max, NSRC, NWIN, wpack, bpack)
    nc = _GRAPH_CACHE[key]

    in_maps = []
    for c in cores:
        in_maps.append({
            "wei": wpack[0], "bia": bpack[0],
            "elenT": c['elenT'], "shT": c['shT'],
            "oheop": c['oheop'], "ohnop": c['ohnop'],
            "srcidx": c['srcidx'], "tsrc": c['table_src'],
            "dstfeat": c['dstfeat'],
        })
    trace = os.environ.get('KTRACE', '0') == '1'
    res = run_bass_kernel_spmd(nc, in_maps, core_ids=list(range(NCORES)),
                               trace=trace)
    if trace:
        print(f"HW exec time: {res.exec_time_ns} ns")

    node_out = np.zeros((N, DOUT), np.float32)
    esc_out = np.zeros((E, 32), np.float32)
    for k, c in enumerate(cores):
        r = res.results[k]
        nreal = min(c['n1'], N) - c['n0']
        node_out[c['n0']:c['n0'] + nreal] = r['nodeout'][:nreal]
        v = c['valid']
        esc_out[c['eid'][v]] = r['esc'].T[v]
    return node_out, esc_out


# revision 10
# speedup vs baseline: 1.0118x; 1.0028x over previous
"""Trainium2 Bass kernel for nn_Block_71554155151896 (GNN message passing).

Sharding: edges sorted by dst on host, split into 8 contiguous 128-aligned
dst-node ranges (one per core). All edges of a node live on one core, so
segment softmax + scatter-sum are core-local (no collectives). The device
does all model math (MLPs, LN, softmax, tensor product, gathers, one-hot
matmul scatter); the host only does index prep / padding / permutation.

Layout: feature-on-partition (fep) [d, edges] for the matmul chain, bf16
matmul inputs with f32 PSUM accumulation. Edges processed per 128-node
window (NWSUB=9 sub-chunks of 128 edges, G=3 matmul groups of 384).
"""
import sys
sys.path.insert(0, "/opt/trn_rl_repo")
import os
import numpy as np
import ml_dtypes
from contextlib import ExitStack

import concourse.bass as bass
import concourse.tile as tile
from concourse import bacc, mybir
from concourse.bass_utils import run_bass_kernel_spmd

bf16 = ml_dtypes.bfloat16
F32 = mybir.dt.float32
BF = mybir.dt.bfloat16
I32 = mybir.dt.int32

N, E, D, S, H, L = 50000, 400000, 64, 9, 4, 64
DOUT = 64
NCORES = 8
P = 128
NWSUB = 9
WE = NWSUB * P        # 1152 edges / window
G = 3
CG = WE // G          # 384
SPG = NWSUB // G      # 3

AF = mybir.ActivationFunctionType
OP = mybir.AluOpType


def _to_bf(x):
    return np.asarray(x, np.float32).astype(bf16)


# ------------------------------------------------------------------ host prep
def preprocess(inputs):
    src = np.asarray(inputs['edge_src'])
    dst = np.asarray(inputs['edge_dst'])
    elen = np.asarray(inputs['edge_length_embedding'])
    sh = np.asarray(inputs['edge_sh'])
    node = np.asarray(inputs['node_in'])

    order = np.argsort(dst, kind='stable')
    dst_s = dst[order]

    bounds = [0]
    for k in range(1, NCORES):
        target = k * E // NCORES
        bounds.append(min((int(dst_s[target]) + 127) // 128 * 128, N))
    bounds.append(((N + 127) // 128) * 128)
    W_max = max((bounds[k + 1] - bounds[k] + 127) // 128 for k in range(NCORES))

    cores = []
    for k in range(NCORES):
        n0, n1 = bounds[k], bounds[k + 1]
        lo = np.searchsorted(dst_s, n0, 'left')
        hi = np.searchsorted(dst_s, min(n1, N), 'left') if n1 <= N else E
        eidx = order[lo:hi]
        dstk = dst[eidx] - n0
        W = (n1 - n0 + 127) // 128

        eid = np.full((W_max, WE), -1, np.int64)
        for w in range(W):
            ids = eidx[(dstk // 128) == w]
            assert len(ids) <= WE, f"window overflow {len(ids)}"
            eid[w, :len(ids)] = ids
        eid = eid.reshape(-1)
        valid = eid >= 0
        eid_c = np.where(valid, eid, 0)
        Ep = eid.size

        uniq, src_rel = np.unique(src[eid_c], return_inverse=True)
        src_rel = np.where(valid, src_rel, len(uniq)).astype(np.int32)
        dst_rel = np.where(valid, dst[eid_c] - n0, 0)
        rel_in_win = dst_rel % 128

        ar = np.arange(128)
        oh_eop = ((rel_in_win[:, None] == ar[None, :]) &
                  valid[:, None]).astype(np.float32)           # [Ep, 128]
        # nop per window: [W_max*128 nodes, WE]
        oh_nop = np.zeros((W_max * 128, WE), bf16)
        for w in range(W_max):
            blk = oh_eop[w * WE:(w + 1) * WE]                  # [WE, 128]
            oh_nop[w * 128:(w + 1) * 128, :] = _to_bf(blk.T)

        table_src = np.zeros((len(uniq) + 128, 64), bf16)
        table_src[:len(uniq)] = _to_bf(node[uniq])

        nreal = min(n1, N) - n0
        dstfeat = np.zeros((W_max * 128, D), bf16)
        dstfeat[:nreal] = _to_bf(node[n0:n0 + nreal])

        elen_T = np.zeros((D, Ep), bf16)
        elen_T[:, valid] = _to_bf(elen[eid_c]).T[:, valid]
        sh_T = np.zeros((S, Ep), bf16)
        sh_T[:, valid] = _to_bf(sh[eid_c]).T[:, valid]

        srcidx = np.zeros((W_max, 128, NWSUB), np.int32)
        sr = src_rel.reshape(W_max, NWSUB, 128)
        srcidx[:] = np.transpose(sr, (0, 2, 1))

        cores.append(dict(
            n0=n0, n1=n1, W=W, eid=eid, valid=valid,
            elenT=elen_T, shT=sh_T, oheop=_to_bf(oh_eop),
            ohnop=oh_nop, srcidx=srcidx,
            table_src=table_src, dstfeat=dstfeat,
        ))
    NSRC = max(c['table_src'].shape[0] for c in cores)
    for c in cores:
        t = c['table_src']
        if t.shape[0] < NSRC:
            c['table_src'] = np.vstack(
                [t, np.zeros((NSRC - t.shape[0], 64), bf16)])
    return cores, dict(W_max=W_max, NSRC=NSRC, bounds=bounds)


def pack_weights(params):
    cols = {}
    buf = []
    pos = 0

    def add(name, mat):
        nonlocal pos
        mat = np.asarray(mat, np.float32)
        m = np.zeros((128, mat.shape[1]), np.float32)
        m[:mat.shape[0]] = mat
        buf.append(m)
        cols[name] = (pos, mat.shape[1])
        pos += mat.shape[1]

    def lhsT_aug(Wm, bm=None):
        Wm = np.asarray(Wm, np.float32)
        rows = [Wm] + ([np.asarray(bm, np.float32)[None, :]] if bm is not None else [])
        Wb = np.vstack(rows)
        return np.concatenate([Wb, Wb.mean(1, keepdims=True)], axis=1)

    p = params
    a, sc, tp2 = p['alpha'], p['scalar'], p['tp2']
    Wp = np.asarray(p['W_pre'])
    add('Wsrc2', np.concatenate([Wp[0:64]] * 2, axis=1))       # [64, 128]
    add('Wdst2', np.concatenate([Wp[64:128]] * 2, axis=1))
    add('Welen2', np.concatenate([Wp[128:192]] * 2, axis=1))
    add('Wa1', lhsT_aug(a['W1'], a['b1']))                     # [65, 65]
    add('Wa2', lhsT_aug(a['W2'], a['b2']))
    add('Wa3', np.vstack([np.asarray(a['W3']),
                          np.asarray(a['b3'])[None, :]]))      # [65, 4]
    add('Wr1', np.asarray(tp2['W_r1']))
    add('Wr2', np.asarray(tp2['W_r2']))                        # [64, 256]
    Wtp = np.asarray(tp2['W_tp'])
    Wflat = np.transpose(Wtp, (1, 0, 2)).reshape(S * D, 256)
    Wflat_p = np.zeros((5 * 128, 256), np.float32)
    Wflat_p[:S * D] = Wflat
    for t in range(5):
        add(f'Wtp{t}', Wflat_p[t * 128:(t + 1) * 128])
    We = np.asarray(p['W_edge'])
    add('WeA', We[0:128]); add('WeB', We[128:256])
    add('Ws1', lhsT_aug(sc['W1'], sc['b1']))
    add('Ws2', lhsT_aug(sc['W2'], sc['b2']))
    add('Ws3', np.vstack([np.asarray(sc['W3']),
                          np.asarray(sc['b3'])[None, :]]))     # [65, 32]
    Wo = np.asarray(p['W_out'])
    add('WoA', Wo[0:128]); add('WoB', Wo[128:256])

    add('ones64', np.full((64, 1), 1.0 / 64.0, np.float32))
    gpatA = np.zeros((1, 128), np.float32); gpatA[0, 0:64] = 1.0
    gpatB = np.zeros((1, 128), np.float32); gpatB[0, 64:128] = 1.0
    add('gpatA', gpatA); add('gpatB', gpatB)
    e4 = np.zeros((4, 256), np.float32)
    for h in range(4):
        e4[h, h * 64:(h + 1) * 64] = 1.0
    add('E4a', e4[:, 0:128]); add('E4b', e4[:, 128:256])
    add('I4', np.eye(4, dtype=np.float32))
    add('I128', np.eye(128, dtype=np.float32))
    for t in range(5):
        sel = np.zeros((S, 128), np.float32)
        for half in range(2):
            srow = 2 * t + half
            if srow < S:
                sel[srow, half * 64:(half + 1) * 64] = 1.0
        add(f'SEL{t}', sel)
    return _to_bf(np.concatenate(buf, axis=1)), cols


def pack_biases(params):
    cols = {}
    buf = []
    tp2 = params['tp2']
    for name, vec in [('br1', tp2['b_r1']),
                      ('br2a', np.asarray(tp2['b_r2'])[0:128]),
                      ('br2b', np.asarray(tp2['b_r2'])[128:256]),
                      ('zero', np.zeros(128))]:
        v = np.zeros((128, 1), np.float32)
        vv = np.asarray(vec, np.float32).ravel()
        v[:len(vv), 0] = vv
        cols[name] = len(buf)
        buf.append(v)
    return np.concatenate(buf, axis=1), cols


# ------------------------------------------------------------------ graph
def build_graph(W_max, NSRC, NWIN, wpack, bpack):
    Wpk, wc = wpack
    bpk, bc = bpack
    Ep = W_max * WE
    nc = bacc.Bacc('TRN2', target_bir_lowering=False, debug=False,
                   num_devices=NCORES)

    d_wei = nc.declare_dram_parameter("wei", list(Wpk.shape), BF, isOutput=False)
    d_bia = nc.declare_dram_parameter("bia", list(bpk.shape), F32, isOutput=False)
    d_elen = nc.declare_dram_parameter("elenT", [D, Ep], BF, isOutput=False)
    d_sh = nc.declare_dram_parameter("shT", [S, Ep], BF, isOutput=False)
    d_oh = nc.declare_dram_parameter("oheop", [Ep, 128], BF, isOutput=False)
    d_ohn = nc.declare_dram_parameter("ohnop", [W_max * 128, WE], BF, isOutput=False)
    d_sidx = nc.declare_dram_parameter("srcidx", [W_max, 128, NWSUB], I32, isOutput=False)
    d_tsrc = nc.declare_dram_parameter("tsrc", [NSRC, 64], BF, isOutput=False)
    d_dstf = nc.declare_dram_parameter("dstfeat", [W_max * 128, D], BF, isOutput=False)

    d_esc = nc.declare_dram_parameter("esc", [32, Ep], F32, isOutput=True)
    d_nout = nc.declare_dram_parameter("nodeout", [W_max * 128, DOUT], F32, isOutput=True)

    def raw_act(out, in_, func, bias, scale=1.0):
        eng = nc.scalar
        inputs = [eng.lower_ap(in_), eng.lower_ap(bias),
                  mybir.ImmediateValue(dtype=F32, value=float(scale)),
                  mybir.ImmediateValue(dtype=F32, value=0.0)]
        return eng.add_instruction(mybir.InstActivation(
            name=nc.get_next_instruction_name(),
            func=func, ins=inputs, outs=[eng.lower_ap(out)]))

    with ExitStack() as ctx:
        tc = ctx.enter_context(tile.TileContext(nc))
        cp = ctx.enter_context(tc.tile_pool(name="const", bufs=1))
        wp = ctx.enter_context(tc.tile_pool(name="win", bufs=3))
        gp = ctx.enter_context(tc.tile_pool(name="grp", bufs=4))
        ps_mlp = ctx.enter_context(tc.tile_pool(name="psm", bufs=2, space="PSUM"))
        ps_ln = ctx.enter_context(tc.tile_pool(name="psl", bufs=2, space="PSUM"))
        ps_ln1 = ctx.enter_context(tc.tile_pool(name="psl1", bufs=1, space="PSUM"))
        ps_tp = ctx.enter_context(tc.tile_pool(name="pst", bufs=2, space="PSUM"))
        ps_big = ctx.enter_context(tc.tile_pool(name="psb", bufs=1, space="PSUM"))

        wei = cp.tile([128, Wpk.shape[1]], BF, tag="wei")
        nc.sync.dma_start(wei[:], d_wei[:])
        bia = cp.tile([128, bpk.shape[1]], F32, tag="bia")
        nc.sync.dma_start(bia[:], d_bia[:])

        def Wt(nm, rows=128):
            o, n = wc[nm]
            return wei[0:rows, o:o + n]

        def Bi(nm, rows=128):
            return bia[0:rows, bc[nm]:bc[nm] + 1]

        nwin = cp.tile([128, W_max, 64], BF, tag="nwin")
        nc.sync.dma_start(
            nwin[:], d_dstf[:].rearrange("(w p) d -> p w d", p=128))

        def ln_silu(x_ps, mlp_tag):
            """x_ps PSUM [65(+), CG]: rows 0:64 = x (incl bias), row 64 = mean.
            Returns SBUF bf16 [65, CG]: rows 0:64 silu(LN(x)), row 64 ones."""
            sq = gp.tile([64, CG], BF, tag="lnsq")
            nc.scalar.activation(sq[:], x_ps[0:64, :], AF.Square)
            msq = ps_ln1.tile([1, CG], F32, tag="lnmsq", space="PSUM")
            nc.tensor.matmul(msq[:], lhsT=Wt('ones64', 64), rhs=sq[:],
                             start=True, stop=True)
            musq = gp.tile([1, CG], F32, tag="lnmusq")
            nc.scalar.activation(musq[:], x_ps[64:65, :], AF.Square)
            var = gp.tile([1, CG], F32, tag="lnvar")
            nc.vector.scalar_tensor_tensor(out=var[:], in0=msq[:], scalar=1e-6,
                                           in1=musq[:], op0=OP.add,
                                           op1=OP.subtract)
            rstd = gp.tile([1, CG], BF, tag="lnrstd")
            raw_act(rstd[:], var[:], AF.Abs_reciprocal_sqrt,
                    bias=Bi('zero', 1))
            murstd = gp.tile([1, CG], BF, tag="lnmurstd")
            nc.vector.tensor_tensor(out=murstd[:], in0=x_ps[64:65, :],
                                    in1=rstd[:], op=OP.mult)
            bcab = ps_ln.tile([128, CG], F32, tag="lnbcab", space="PSUM")
            nc.tensor.matmul(bcab[:], lhsT=Wt('gpatA', 1), rhs=rstd[:],
                             start=True, stop=False)
            nc.tensor.matmul(bcab[:], lhsT=Wt('gpatB', 1), rhs=murstd[:],
                             start=False, stop=True)
            bcsa = gp.tile([64, CG], BF, tag="lnbcsa")
            nc.vector.tensor_copy(bcsa[:], bcab[0:64, :])
            bcsb = gp.tile([64, CG], BF, tag="lnbcsb")
            nc.scalar.copy(bcsb[:], bcab[64:128, :])
            t1 = gp.tile([64, CG], BF, tag="lnt1")
            nc.vector.tensor_tensor(out=t1[:], in0=x_ps[0:64, :],
                                    in1=bcsa[:], op=OP.mult)
            t2 = gp.tile([64, CG], BF, tag="lnt2")
            nc.vector.tensor_tensor(out=t2[:], in0=t1[:], in1=bcsb[:],
                                    op=OP.subtract)
            out = gp.tile([65, CG], BF, tag="ln_" + mlp_tag)
            nc.vector.memset(out[64:65, :], 1.0)
            nc.scalar.activation(out[0:64, :], t2[:], AF.Silu)
            return out

        for w in range(NWIN):
            e0 = w * WE
            elw = wp.tile([65, WE], BF, tag="elw")
            nc.sync.dma_start(elw[0:64, :], d_elen[:, e0:e0 + WE])
            nc.vector.memset(elw[64:65, :], 1.0)
            shw = wp.tile([S, WE], BF, tag="shw")
            nc.sync.dma_start(shw[:], d_sh[:, e0:e0 + WE])
            ohw = wp.tile([128, NWSUB, 128], BF, tag="ohw")
            nc.sync.dma_start(
                ohw[:], d_oh[e0:e0 + WE, :].rearrange("(s p) n -> p s n", p=128))
            ohn = wp.tile([128, WE], BF, tag="ohn")
            nc.sync.dma_start(ohn[:], d_ohn[w * 128:(w + 1) * 128, :])

            # ---- pass A: alpha logits -> ex -> den ----
            exw = wp.tile([4, WE], BF, tag="exw")
            for g in range(G):
                c0 = g * CG
                x1 = ps_mlp.tile([65, CG], F32, tag="mlp", space="PSUM")
                nc.tensor.matmul(x1[:], lhsT=Wt('Wa1', 65),
                                 rhs=elw[:, c0:c0 + CG], start=True, stop=True)
                h1 = ln_silu(x1, "a1")
                x2 = ps_mlp.tile([65, CG], F32, tag="mlp", space="PSUM")
                nc.tensor.matmul(x2[:], lhsT=Wt('Wa2', 65), rhs=h1[:],
                                 start=True, stop=True)
                h2 = ln_silu(x2, "a2")
                lg = ps_mlp.tile([4, CG], F32, tag="mlp", space="PSUM")
                nc.tensor.matmul(lg[:], lhsT=Wt('Wa3', 65), rhs=h2[:],
                                 start=True, stop=True)
                nc.scalar.activation(exw[:, c0:c0 + CG], lg[:], AF.Exp)

            exe_ps = ps_big.tile([128, NWSUB * 4], F32, tag="big", space="PSUM")
            for s in range(NWSUB):
                nc.tensor.matmul(exe_ps[:, s * 4:(s + 1) * 4],
                                 lhsT=exw[:, s * P:(s + 1) * P],
                                 rhs=Wt('I4', 4), start=True, stop=True)
            exe = wp.tile([128, NWSUB * 4], BF, tag="exe")
            nc.scalar.copy(exe[:], exe_ps[:])

            den_ps = ps_ln1.tile([128, 4], F32, tag="lnmsq", space="PSUM")
            for s in range(NWSUB):
                nc.tensor.matmul(den_ps[:], lhsT=ohw[:, s, :],
                                 rhs=exe[:, s * 4:(s + 1) * 4],
                                 start=(s == 0), stop=(s == NWSUB - 1))
            dent = wp.tile([128, 4], F32, tag="dent")
            nc.vector.tensor_scalar(out=dent[:], in0=den_ps[:], scalar1=1e-12,
                                    scalar2=None, op0=OP.add)
            dent2 = wp.tile([128, 4], F32, tag="dent2")
            nc.vector.reciprocal(out=dent2[:], in_=dent[:])
            denw = wp.tile([128, 4], BF, tag="denw")
            nc.scalar.copy(denw[:], dent2[:])

            # ---- src gather ----
            srcw = wp.tile([128, NWSUB, 64], BF, tag="srcw")
            sidx = wp.tile([128, NWSUB], I32, tag="sidx")
            nc.sync.dma_start(sidx[:], d_sidx[w])
            for s in range(NWSUB):
                nc.gpsimd.indirect_dma_start(
                    out=srcw[:, s, :], out_offset=None, in_=d_tsrc[:],
                    in_offset=bass.IndirectOffsetOnAxis(
                        ap=sidx[:, s:s + 1], axis=0))

            # ---- per-group value chain ----
            acc = wp.tile([128, DOUT], F32, tag="acc")
            nc.vector.memset(acc[:], 0.0)

            for g in range(G):
                c0 = g * CG
                # srcT fep
                srcT_ps = ps_ln.tile([64, CG], F32, tag="lnbcab", space="PSUM")
                for j in range(SPG):
                    s = g * SPG + j
                    nc.tensor.matmul(srcT_ps[:, j * P:(j + 1) * P],
                                     lhsT=srcw[:, s, :], rhs=Wt('I128'),
                                     start=True, stop=True)
                srcT = gp.tile([64, CG], BF, tag="srcT")
                nc.vector.tensor_copy(srcT[:], srcT_ps[:])

                # dst features + recip den per edge (one MM per group)
                dstf_ps = ps_ln.tile([64, CG], F32, tag="lnbcab", space="PSUM")
                nc.tensor.matmul(dstf_ps[:], lhsT=nwin[:, w, :],
                                 rhs=ohn[:, c0:c0 + CG], start=True, stop=True)
                dstT = gp.tile([64, CG], BF, tag="dstT")
                nc.vector.tensor_copy(dstT[:], dstf_ps[:])
                dden_ps = ps_mlp.tile([4, CG], F32, tag="mlp", space="PSUM")
                nc.tensor.matmul(dden_ps[:], lhsT=denw[:],
                                 rhs=ohn[:, c0:c0 + CG], start=True, stop=True)
                dden = gp.tile([4, CG], BF, tag="dden")
                nc.scalar.copy(dden[:], dden_ps[:])

                # msg (stacked x2)
                msg_ps = ps_big.tile([128, CG], F32, tag="big", space="PSUM")
                nc.tensor.matmul(msg_ps[:], lhsT=Wt('Wsrc2', 64), rhs=srcT[:],
                                 start=True, stop=False)
                nc.tensor.matmul(msg_ps[:], lhsT=Wt('Wdst2', 64),
                                 rhs=dstT[:], start=False, stop=False)
                nc.tensor.matmul(msg_ps[:], lhsT=Wt('Welen2', 64),
                                 rhs=elw[0:64, c0:c0 + CG], start=False, stop=True)
                msg2 = gp.tile([128, CG], BF, tag="msg2")
                nc.scalar.copy(msg2[:], msg_ps[:])

                # alpha = ex * recip_den[dst]
                alpha4 = gp.tile([4, CG], BF, tag="alpha4")
                nc.vector.tensor_tensor(out=alpha4[:], in0=exw[:, c0:c0 + CG],
                                        in1=dden[:], op=OP.mult)
                aexp_ps = ps_tp.tile([128, CG], F32, tag="tp", space="PSUM")
                nc.tensor.matmul(aexp_ps[:], lhsT=Wt('E4a', 4), rhs=alpha4[:],
                                 start=True, stop=True)
                aexp_ps2 = ps_tp.tile([128, CG], F32, tag="tp", space="PSUM")
                nc.tensor.matmul(aexp_ps2[:], lhsT=Wt('E4b', 4), rhs=alpha4[:],
                                 start=True, stop=True)
                aexp = gp.tile([128, 2, CG], BF, tag="aexp")
                nc.scalar.copy(aexp[:, 0, :], aexp_ps[:])
                nc.scalar.copy(aexp[:, 1, :], aexp_ps2[:])

                # radial, radalpha = (radial + br2) * alphaexp
                r1_ps = ps_mlp.tile([64, CG], F32, tag="mlp", space="PSUM")
                nc.tensor.matmul(r1_ps[:], lhsT=Wt('Wr1', 64),
                                 rhs=elw[0:64, c0:c0 + CG], start=True, stop=True)
                r1 = gp.tile([64, CG], BF, tag="r1")
                nc.scalar.activation(r1[:], r1_ps[:], AF.Silu, bias=Bi('br1', 64))
                radal = gp.tile([128, 2, CG], BF, tag="radal")
                for half in range(2):
                    rad_ps = ps_big.tile([128, CG], F32, tag="big", space="PSUM")
                    nc.tensor.matmul(
                        rad_ps[:], lhsT=Wt('Wr2', 64)[:, half * 128:(half + 1) * 128],
                        rhs=r1[:], start=True, stop=True)
                    nc.vector.scalar_tensor_tensor(
                        out=radal[:, half, :], in0=rad_ps[:],
                        scalar=Bi('br2a' if half == 0 else 'br2b'),
                        in1=aexp[:, half, :], op0=OP.add, op1=OP.mult)

                # kron + tensor product
                tpA = ps_tp.tile([128, CG], F32, tag="tp", space="PSUM")
                tpB = ps_tp.tile([128, CG], F32, tag="tp", space="PSUM")
                for t in range(5):
                    shp_ps = ps_big.tile([128, CG], F32, tag="big", space="PSUM")
                    nc.tensor.matmul(shp_ps[:], lhsT=Wt(f'SEL{t}', S),
                                     rhs=shw[:, c0:c0 + CG], start=True, stop=True)
                    shp = gp.tile([128, CG], BF, tag="shp")
                    nc.vector.tensor_copy(shp[:], shp_ps[:])
                    kro = gp.tile([128, CG], BF, tag="kro")
                    nc.vector.tensor_tensor(out=kro[:], in0=msg2[:], in1=shp[:],
                                            op=OP.mult)
                    nc.tensor.matmul(tpA[:], lhsT=Wt(f'Wtp{t}')[:, 0:128],
                                     rhs=kro[:], start=(t == 0), stop=(t == 4))
                    nc.tensor.matmul(tpB[:], lhsT=Wt(f'Wtp{t}')[:, 128:256],
                                     rhs=kro[:], start=(t == 0), stop=(t == 4))

                eo = gp.tile([128, 2, CG], BF, tag="eo")
                nc.vector.tensor_tensor(out=eo[:, 0, :], in0=tpA[:],
                                        in1=radal[:, 0, :], op=OP.mult)
                nc.vector.tensor_tensor(out=eo[:, 1, :], in0=tpB[:],
                                        in1=radal[:, 1, :], op=OP.mult)

                # edge scalar head
                xes_ps = ps_ln.tile([64, CG], F32, tag="lnbcab", space="PSUM")
                nc.tensor.matmul(xes_ps[:], lhsT=Wt('WeA'), rhs=eo[:, 0, :],
                                 start=True, stop=False)
                nc.tensor.matmul(xes_ps[:], lhsT=Wt('WeB'), rhs=eo[:, 1, :],
                                 start=False, stop=True)
                xes = gp.tile([65, CG], BF, tag="xes")
                nc.vector.memset(xes[64:65, :], 1.0)
                nc.scalar.copy(xes[0:64, :], xes_ps[:])
                xs1 = ps_mlp.tile([65, CG], F32, tag="mlp", space="PSUM")
                nc.tensor.matmul(xs1[:], lhsT=Wt('Ws1', 65), rhs=xes[:],
                                 start=True, stop=True)
                hs1 = ln_silu(xs1, "s1")
                xs2 = ps_mlp.tile([65, CG], F32, tag="mlp", space="PSUM")
                nc.tensor.matmul(xs2[:], lhsT=Wt('Ws2', 65), rhs=hs1[:],
                                 start=True, stop=True)
                hs2 = ln_silu(xs2, "s2")
                esc_ps = ps_ln.tile([32, CG], F32, tag="lnbcab", space="PSUM")
                nc.tensor.matmul(esc_ps[:], lhsT=Wt('Ws3', 65), rhs=hs2[:],
                                 start=True, stop=True)
                escs = gp.tile([32, CG], F32, tag="escs")
                nc.scalar.copy(escs[:], esc_ps[:])
                nc.sync.dma_start(d_esc[:, e0 + c0:e0 + c0 + CG], escs[:])

                # proj + scatter
                prT_ps = ps_ln.tile([64, CG], F32, tag="lnbcab", space="PSUM")
                nc.tensor.matmul(prT_ps[:], lhsT=Wt('WoA'), rhs=eo[:, 0, :],
                                 start=True, stop=False)
                nc.tensor.matmul(prT_ps[:], lhsT=Wt('WoB'), rhs=eo[:, 1, :],
                                 start=False, stop=True)
                prT = gp.tile([64, CG], BF, tag="prT")
                nc.scalar.copy(prT[:], prT_ps[:])
                pre_ps = ps_big.tile([128, SPG * 64], F32, tag="big", space="PSUM")
                for j in range(SPG):
                    nc.tensor.matmul(pre_ps[:, j * 64:(j + 1) * 64],
                                     lhsT=prT[:, j * P:(j + 1) * P],
                                     rhs=Wt('I128', 64)[:, 0:64],
                                     start=True, stop=True)
                pre = gp.tile([128, SPG * 64], BF, tag="pre")
                nc.vector.tensor_copy(pre[:], pre_ps[:])
                accg = ps_ln.tile([128, 64], F32, tag="lnbcab", space="PSUM")
                for j in range(SPG):
                    s = g * SPG + j
                    nc.tensor.matmul(accg[:], lhsT=ohw[:, s, :],
                                     rhs=pre[:, j * 64:(j + 1) * 64],
                                     start=(j == 0), stop=(j == SPG - 1))
                nc.vector.tensor_add(acc[:], acc[:], accg[:])

            nc.sync.dma_start(d_nout[w * 128:(w + 1) * 128, :], acc[:])

    nc.compile()
    return nc


# ------------------------------------------------------------------ driver
_GRAPH_CACHE = {}


def kernel(node_in, node_embed, edge_sh, edge_length_embedding,
           edge_src, edge_dst, batch, params):
    inputs = dict(node_in=node_in, node_embed=node_embed, edge_sh=edge_sh,
                  edge_length_embedding=edge_length_embedding,
                  edge_src=edge_src, edge_dst=edge_dst, batch=batch,
                  params=params)
    cores, meta = preprocess(inputs)
    W_max, NSRC = meta['W_max'], meta['NSRC']
    wpack = pack_weights(params)
    bpack = pack_biases(params)

    NWIN = int(os.environ.get('KNWIN', W_max))
    key = (W_max, NSRC, NWIN)
    if key not in _GRAPH_CACHE:
        _GRAPH_CACHE[key] = build_graph(W_max, NSRC, NWIN, wpack, bpack)
    nc = _GRAPH_CACHE[key]

    in_maps = []
    for c in cores:
        in_maps.append({
            "wei": wpack[0], "bia": bpack[0],
            "elenT": c['elenT'], "shT": c['shT'],
            "oheop": c['oheop'], "ohnop": c['ohnop'],
            "srcidx": c['srcidx'], "tsrc": c['table_src'],
            "dstfeat": c['dstfeat'],
        })
    trace = os.environ.get('KTRACE', '0') == '1'
    res = run_bass_kernel_spmd(nc, in_maps, core_ids=list(range(NCORES)),
                               trace=trace)
    if trace:
        print(f"HW exec time: {res.exec_time_ns} ns")

    node_out = np.zeros((N, DOUT), np.float32)
    esc_out = np.zeros((E, 32), np.float32)
    for k, c in enumerate(cores):
        r = res.results[k]
        nreal = min(c['n1'], N) - c['n0']
        node_out[c['n0']:c['n0'] + nreal] = r['nodeout'][:nreal]
        v = c['valid']
        esc_out[c['eid'][v]] = r['esc'].T[v]
    return node_out, esc_out
